# revision 1
# baseline (speedup 1.0000x reference)
"""Multi-head attention (B=4, S=2048, D=1280, H=10, hd=128) on 8 TRN2 NeuronCores.

Sharding: core c handles batch b = c//2 and heads h0 = 5*(c%2) .. h0+5
(data-parallel over batch x head-parallel tensor parallelism). Host does the
final pairwise all-reduce + bias.

Precision/speed scheme (PE is the bottleneck engine):
  - Projections run as fp8 DoubleRow matmuls (2 contraction k-tiles per pass,
    0.5 cyc/row) with *residual compensation*: operands split hi+lo in e4m3 at
    a fixed power-of-2 scale, cross terms accumulated in fp32 PSUM. V and K
    use 3 terms (x_hi*w_hi + x_lo*w_hi + x_hi*w_lo, ~bf16-grade); Q uses
    2 terms (x compensated, w_q plain fp8) since Q is re-quantized to fp8
    anyway and its noise is dominated by that store.
  - Q,K are stored e4m3 (8x true scale) in the [64, 2, S] split-hd layout
    DoubleRow wants; S^T = K Q^T then runs fp8-DoubleRow at half bf16 cost.
  - P = exp(S*scale) stays bf16 (scale folds all fp8 scaling), O = P V and
    the out-projection stay bf16: their quantization noise would not average
    down (attention output is itself a near-uniform average).
  - V is projected directly transposed (stationary x-tiles, moving w_v),
    removing the baseline's 80 PE transposes and its ACT copy chain.

Schedule: input DMAs all on the SP ring in consumption order; K-proj chunks
interleaved with V-proj token-tiles to track the x stream; Q chunk 0; then
per (chunk, head) attention units with the baseline's jp software pipeline.
The out-projection of chunk ic-1 and the Q-projection of chunk ic+1 run as
PE filler inside the units. Q/K quantization: DVE writes an fp8 staging tile;
two small SBUF->SBUF DMAs on the scalar-HWDGE and Pool-SWDGE rings (kept free
of bulk traffic) move the halves into the [64, 2, ...] layout.
"""

import numpy as np

B, S, D = 4, 2048, 1280
HEADS = 10
HD = 128
NH = 5              # heads per core
P = 128
SCALE = float(D) ** -0.5
KT_D = D // P       # 10 k-tiles over D
KTP = KT_D // 2     # 5 DoubleRow k-tile pairs
NJT = S // P        # 16 j tiles
NIC = S // 512      # 4 i-chunks of 512
CX = 4.0            # x fp8 scale
CW = 64.0           # w fp8 scale
CQK = 1.0 / 32.0    # Q/K store rescale: psum 256x -> stored 8x true
EXP_SCALE = SCALE / 64.0   # dots psum carries (8*8)=64x true scale
CV = 1.0 / 256.0    # V store rescale: psum 256x -> true

_PROGRAM_CACHE = {}


def _build_program(repeat=1):
    if repeat in _PROGRAM_CACHE:
        return _PROGRAM_CACHE[repeat]

    import concourse.mybir as mybir
    from concourse import bacc
    import concourse.tile as tile

    F32 = mybir.dt.float32
    F32R = mybir.dt.float32r
    BF16 = mybir.dt.bfloat16
    F8 = mybir.dt.float8e4
    EXP = mybir.ActivationFunctionType.Exp
    DR = mybir.MatmulPerfMode.DoubleRow

    nc = bacc.Bacc()
    xh_d = nc.declare_dram_parameter("xh", [D, S], F8, isOutput=False)
    xl_d = nc.declare_dram_parameter("xl", [D, S], F8, isOutput=False)
    wqkh_d = nc.declare_dram_parameter("wqkh", [D, 2 * NH * HD], F8, isOutput=False)
    wvh_d = nc.declare_dram_parameter("wvh", [D, NH * HD], F8, isOutput=False)
    wvl_d = nc.declare_dram_parameter("wvl", [D, NH * HD], F8, isOutput=False)
    wout_d = nc.declare_dram_parameter("wout", [NH * HD, D], BF16, isOutput=False)
    onesr_d = nc.declare_dram_parameter("onesr_in", [1, P], F32, isOutput=False)
    out_d = nc.declare_dram_parameter("outT", [D, S], F32, isOutput=True)

    xh_t = xh_d[:].rearrange("(kt p) s -> p kt s", p=P)        # [128, 10, 2048]
    xl_t = xl_d[:].rearrange("(kt p) s -> p kt s", p=P)
    wqkh_t = wqkh_d[:].rearrange("(kt p) m -> p kt m", p=P)    # [128, 10, 1280]
    wvh_t = wvh_d[:].rearrange("(kt p) m -> p kt m", p=P)
    wvl_t = wvl_d[:].rearrange("(kt p) m -> p kt m", p=P)
    wout_t = wout_d[:].rearrange("(kt p) m -> p kt m", p=P)    # [128, 5, 1280]

    with tile.TileContext(nc) as tc:
        with (
            tc.tile_pool(name="persist", bufs=1) as persist,
            tc.tile_pool(name="oio", bufs=3) as oio,
            tc.tile_pool(name="work", bufs=4) as work,
            tc.tile_pool(name="ptp", bufs=6) as ptp,
            tc.tile_pool(name="work2", bufs=2) as work2,
            tc.tile_pool(name="stgp", bufs=6) as stgp,
            tc.tile_pool(name="ps_mm", bufs=2, space="PSUM") as ps_mm,
            tc.tile_pool(name="ps_acc", bufs=2, space="PSUM") as ps_acc,
            tc.tile_pool(name="ps_sm", bufs=2, space="PSUM") as ps_sm,
        ):
            XH = persist.tile([P, KT_D, S], F8, name="XH")
            XL = persist.tile([P, KT_D, S], F8, name="XL")
            WQKH = persist.tile([P, KT_D, 2 * NH * HD], F8, name="WQKH")
            WVH = persist.tile([P, KT_D, NH * HD], F8, name="WVH")
            WVL = persist.tile([P, KT_D, NH * HD], F8, name="WVL")
            WO = persist.tile([P, NH, D], BF16, name="WO")
            QS = persist.tile([64, 2, NH, S], F8, name="QS")
            KS = persist.tile([64, 2, NH, S], F8, name="KS")
            V = persist.tile([P, NJT, NH * HD], BF16, name="V")
            ones = persist.tile([P, 1], BF16, name="ones")
            onesr = persist.tile([1, P], F32R, name="onesr")

            scr = persist.tile([P, 1], BF16, name="scr")
            nc.gpsimd.memset(ones[:], 1.0)
            # dummy exp: forces the Exp table load while ACT is idle, so the
            # first attention unit's exp doesn't pay the ~1.3us load
            nc.scalar.activation(scr[:], ones[:], EXP, scale=1.0)

            def load_inputs():
                # Everything on the SP ring, in consumption order, so the
                # (globally serialized) DMA engines feed the lead-in without
                # the small latency-critical shift DMAs queueing behind bulk.
                def w_m(m):
                    msl = slice(m * P, (m + 1) * P)
                    nc.sync.dma_start(WQKH[:, :, msl], wqkh_t[:, :, msl])

                def x_ic(ic):
                    isl = slice(ic * 512, (ic + 1) * 512)
                    nc.sync.dma_start(XH[:, :, isl], xh_t[:, :, isl])
                    nc.sync.dma_start(XL[:, :, isl], xl_t[:, :, isl])

                nc.sync.dma_start(onesr[:], onesr_d[:].bitcast(F32R))
                w_m(NH)
                x_ic(0)
                nc.sync.dma_start(WVH[:], wvh_t)
                nc.sync.dma_start(WVL[:], wvl_t)
                for m in range(1, NH):
                    w_m(NH + m)
                x_ic(1)
                x_ic(2)
                for m in range(NH):
                    w_m(m)       # Q columns
                x_ic(3)
                nc.sync.dma_start(WO[:], wout_t)

            def proj_qk(m, ic, dst, h, main=False):
                """One [128,512] Q or K projection tile -> fp8 into dst[64,2,...].

                K (dst is KS) adds the w_lo cross term; Q skips it."""
                isl = slice(ic * 512, (ic + 1) * 512)
                msl = slice(m * P, (m + 1) * P)
                terms = ((WQKH, msl, XH), (WQKH, msl, XL))
                q_ps = ps_sm.tile([P, 512], F32, name="sm")
                nterm = len(terms) * KTP
                step = 0
                for ktp in range(KTP):
                    k2 = slice(2 * ktp, 2 * ktp + 2)
                    for Wt, wsl, Xt in terms:
                        nc.tensor.matmul(
                            q_ps[:], Wt[:, k2, wsl], Xt[:, k2, isl],
                            start=(step == 0), stop=(step == nterm - 1),
                            perf_mode=DR,
                        )
                        step += 1
                stg = stgp.tile([P, 512], F8, name="stg")
                nc.vector.tensor_scalar_mul(stg[:], q_ps[:], CQK)
                # layout-shift DMAs ride HWDGE rings only (SWDGE via the
                # Pool engine shows a ~10x slowdown red flag on real HW).
                # In the main loop the SP ring is free of bulk traffic.
                lo_ring = nc.sync if main else nc.scalar
                lo_ring.dma_start(dst[:, 0, h, isl], stg[0:64, :])
                nc.scalar.dma_start(dst[:, 1, h, isl], stg[64:128, :])

            def proj_v(tt):
                """V rows for token-tile tt, direct-transposed: [128 tok, 640]."""
                tsl = slice(tt * P, (tt + 1) * P)
                v_ps = ps_mm.tile([P, 1024], F32, name="mm")
                step = 0
                for ktp in range(KTP):
                    k2 = slice(2 * ktp, 2 * ktp + 2)
                    for Xt, Wt in ((XH, WVH), (XH, WVL), (XL, WVH)):
                        st = (step == 0)
                        sp = (step == 3 * KTP - 1)
                        nc.tensor.matmul(v_ps[:, 0:512], Xt[:, k2, tsl],
                                         Wt[:, k2, 0:512], start=st, stop=sp,
                                         perf_mode=DR)
                        nc.tensor.matmul(v_ps[:, 512:640], Xt[:, k2, tsl],
                                         Wt[:, k2, 512:640], start=st, stop=sp,
                                         perf_mode=DR)
                        step += 1
                nc.vector.tensor_scalar_mul(V[:, tt], v_ps[:, 0:640], CV)

            for rep in range(repeat):
                if rep == 0:
                    load_inputs()

                # ---- lead-in: K chunks + V token-tiles track the x stream.
                # 2-term K outruns the DMA feed, so V groups (slower per
                # x-byte) slot between chunks to absorb stream latency ----
                for m in range(NH):
                    proj_qk(NH + m, 0, KS, m)
                for tt in range(4):
                    proj_v(tt)
                for m in range(NH):
                    proj_qk(NH + m, 1, KS, m)
                for tt in range(4, 8):
                    proj_v(tt)
                for m in range(NH):
                    proj_qk(NH + m, 2, KS, m)
                for tt in range(8, 12):
                    proj_v(tt)
                for m in range(NH):
                    proj_qk(NH + m, 3, KS, m)
                for m in range(NH):
                    proj_qk(m, 0, QS, m)
                for tt in range(12, NJT):
                    proj_v(tt)

                # ---- attention + out projection -----------------------------
                def norm_tail_a(st):
                    """colsum: fold halves -> ones-matmul into PSUM."""
                    fold, o_ps, OT, h = st
                    fh = work2.tile([P, 512], BF16, name="fh", tag="fh")
                    nc.vector.tensor_add(fh[:], fold[:, :512], fold[:, 512:])
                    sum_ps = ps_sm.tile([P, 512], F32, name="sm")[0:1, :]
                    nc.tensor.matmul(sum_ps, ones[:], fh[:],
                                     start=True, stop=True)
                    s_row = work2.tile([1, 512], F32R, name="s_row", tag="s_row")
                    nc.vector.tensor_copy(s_row[:], sum_ps)
                    return (s_row, o_ps, OT, h)

                def norm_tail_b(st):
                    """broadcast + reciprocal + normalize into OT."""
                    s_row, o_ps, OT, h = st
                    bc_ps = ps_sm.tile([P, 512], F32, name="sm")
                    nc.tensor.matmul(bc_ps[:], onesr[:], s_row[:],
                                     start=True, stop=True)
                    rec = work2.tile([P, 512], F32, name="rec", tag="rec")
                    nc.vector.reciprocal(rec[:], bc_ps[:])
                    nc.vector.tensor_mul(OT[:, h, :], o_ps[:], rec[:])

                def norm_tail(st):
                    norm_tail_b(norm_tail_a(st))

                def out_proj(ic, OT, ms, alt=False):
                    isl = slice(ic * 512, (ic + 1) * 512)
                    for i, m in enumerate(ms):
                        p_ps = ps_sm.tile([P, 512], F32, name="sm")
                        for kt in range(NH):
                            nc.tensor.matmul(
                                p_ps[:], WO[:, kt, m * P:(m + 1) * P], OT[:, kt, :],
                                start=(kt == 0), stop=(kt == NH - 1),
                            )
                        outc = work.tile([P, 512], F32, name="outc")
                        nc.vector.tensor_copy(outc[:], p_ps[:])
                        nc.sync.dma_start(out_d[m * P:(m + 1) * P, isl], outc[:])

                pending_tail = None
                pending_proj = None
                tail_mid = None
                for ic in range(NIC):
                    isl = slice(ic * 512, (ic + 1) * 512)
                    OT = oio.tile([P, NH, 512], BF16, name="OT")
                    for h in range(NH):
                        fold = work2.tile([P, 1024], BF16, name="fold", tag="fold")
                        o_ps = ps_acc.tile([P, 512], F32, name="acc")
                        pt2s = [None] * (NJT // 2)
                        # software-pipelined: the paired S-DR-matmuls + one
                        # wide exp run a pair ahead of the O-matmuls so PE
                        # never waits on ACT.
                        for jp in range(NJT // 2 + 2):
                            if jp < NJT // 2:
                                s_ps = ps_mm.tile([P, 1024], F32, name="mm")
                                for half in range(2):
                                    jt = 2 * jp + half
                                    jsl = slice(jt * P, (jt + 1) * P)
                                    nc.tensor.matmul(
                                        s_ps[:, half * 512:(half + 1) * 512],
                                        KS[:, :, h, jsl], QS[:, :, h, isl],
                                        start=True, stop=True, perf_mode=DR,
                                    )
                                pt2 = ptp.tile([P, 1024], BF16, name="pt")
                                nc.scalar.activation(pt2[:], s_ps[:], EXP,
                                                     scale=EXP_SCALE)
                                pt2s[jp] = pt2
                                if jp == 1:
                                    nc.vector.tensor_add(
                                        fold[:], pt2s[0][:], pt2s[1][:])
                                elif jp > 1:
                                    nc.vector.tensor_add(fold[:], fold[:], pt2[:])
                            if jp > 1:
                                prev = pt2s[jp - 2]
                                for half in range(2):
                                    jt = 2 * (jp - 2) + half
                                    nc.tensor.matmul(
                                        o_ps[:], V[:, jt, h * P:(h + 1) * P],
                                        prev[:, half * 512:(half + 1) * 512],
                                        start=(jt == 0), stop=(jt == NJT - 1),
                                    )
                            if jp == 0 and pending_tail is not None:
                                tail_mid = norm_tail_a(pending_tail)
                                pending_tail = None
                            if jp == 2 and tail_mid is not None:
                                norm_tail_b(tail_mid)
                                tail_mid = None
                            if jp in (4, 6) and pending_proj is not None:
                                pic, pOT = pending_proj
                                m0 = 2 * h + (0 if jp == 4 else 1)
                                out_proj(pic, pOT, [m0], alt=(jp == 6))
                                if h == NH - 1 and jp == 6:
                                    pending_proj = None
                            if jp == 5 and ic < NIC - 1:
                                # Q projection of the next chunk as PE filler
                                proj_qk(h, ic + 1, QS, h, main=True)
                        pending_tail = (fold, o_ps, OT, h)
                    pending_proj = (ic, OT)
                norm_tail(pending_tail)
                out_proj(*pending_proj, range(D // P))

    nc.finalize()
    _PROGRAM_CACHE[repeat] = nc
    return nc


def _enc_hi_lo(a, scale):
    """Split scale*a into e4m3 hi + lo (same scale; lo holds the residual)."""
    import ml_dtypes
    f8 = ml_dtypes.float8_e4m3
    sa = np.asarray(a, np.float32) * scale
    hi = sa.astype(f8)
    lo = (sa - hi.astype(np.float32)).astype(f8)
    return hi, lo


def _shard_inputs(x, w_qkv, w_out):
    """Build the 8 per-core input maps (fp8 hi/lo operands, host-encoded)."""
    import ml_dtypes
    bf16 = ml_dtypes.bfloat16
    onesr = np.ones((1, P), np.float32)
    in_maps = []
    for c in range(8):
        b = c // 2
        h0 = NH * (c % 2)
        xT = np.ascontiguousarray(np.asarray(x[b], np.float32).T)      # [D, S]
        xh, xl = _enc_hi_lo(xT, CX)
        qk = np.concatenate([
            w_qkv[:, qi * D + h0 * HD: qi * D + (h0 + NH) * HD] for qi in range(2)
        ], axis=1)                                                     # [D, 1280]
        wqkh, _ = _enc_hi_lo(qk, CW)   # 2-term: w_qk plain fp8
        wv = w_qkv[:, 2 * D + h0 * HD: 2 * D + (h0 + NH) * HD]         # [D, 640]
        wvh, wvl = _enc_hi_lo(wv, CW)
        in_maps.append(dict(
            xh=xh, xl=xl, wqkh=wqkh,
            wvh=wvh, wvl=wvl,
            wout=np.ascontiguousarray(
                np.asarray(w_out[h0 * HD:(h0 + NH) * HD, :], np.float32)
            ).astype(bf16),
            onesr_in=onesr,
        ))
    return in_maps


def run_sharded(x, w_qkv, w_out, b_out, repeat=1, trace=False):
    """Run the SPMD program; returns (out [B,S,D], BassKernelResults)."""
    from concourse.bass_utils import run_bass_kernel_spmd

    nc = _build_program(repeat)
    in_maps = _shard_inputs(x, w_qkv, w_out)
    res = run_bass_kernel_spmd(nc, in_maps, list(range(8)), trace=trace)
    out = np.empty((B, S, D), np.float32)
    for b in range(B):
        out[b] = (res.results[2 * b]["outT"].T
                  + res.results[2 * b + 1]["outT"].T
                  + b_out[None, :])
    return out, res


def kernel(x, w_qkv, w_out, b_out):
    x = np.asarray(x, np.float32)
    w_qkv = np.asarray(w_qkv, np.float32)
    w_out = np.asarray(w_out, np.float32)
    b_out = np.asarray(b_out, np.float32)
    out, _ = run_sharded(x, w_qkv, w_out, b_out)
    return out



# revision 2
# speedup vs baseline: 1.0393x; 1.0393x over previous
"""Multi-head attention (B=4, S=2048, D=1280, H=10, hd=128) on 8 TRN2 NeuronCores.

Sharding: core c handles batch b = c//2 and heads h0 = 5*(c%2) .. h0+5
(data-parallel over batch x head-parallel tensor parallelism). Host does the
final pairwise all-reduce + bias.

Precision/speed scheme (PE is the bottleneck engine):
  - Projections run as fp8 DoubleRow matmuls (2 contraction k-tiles per pass,
    0.5 cyc/row) with *residual compensation*: operands split hi+lo in e4m3 at
    a fixed power-of-2 scale, cross terms accumulated in fp32 PSUM. V uses
    3 terms (x_hi*w_hi + x_lo*w_hi + x_hi*w_lo, ~bf16-grade); Q uses 2 terms
    (x compensated, w plain fp8); K uses 1 term (x_hi*w_hi only) - both are
    re-quantized to e4m3 for storage anyway, so the fp8 store noise dominates
    and the compensation terms beyond these don't move the end-to-end error.
  - Q,K are stored e4m3 (8x true scale) in the [64, 2, S] split-hd layout
    DoubleRow wants; S^T = K Q^T then runs fp8-DoubleRow at half bf16 cost.
  - P = exp(S*scale) stays bf16 (scale folds all fp8 scaling).  O = P V runs
    bf16; the normalized attention output OT is kept in fp32 and fed to the
    out-projection as an f32r moving operand (same 1 cyc/row as bf16 for
    free-dim >= 256), so the only out-projection noise is the bf16 W_out.
  - V is projected directly transposed (stationary x-tiles, moving w_v).
  - Softmax colsum: DVE folds P pairs to one [128,512] tile, then a single
    all-ones [128,128] stationary matmul produces the column sums already
    broadcast across all 128 partitions (one 512-row pass; no separate
    [1,512] sum + re-broadcast chain).

Schedule: x/weight DMAs split across the two HWDGE rings (SP: XH + K-column
weights + Q columns; ACT: XL + V weights + WO) in consumption order. K-proj
(1-term, XH-only) tracks the XH stream; V-proj token-tiles follow as both
streams land; Q chunk 0 last; then per (chunk, head) attention units with the
jp software pipeline. The out-projection of chunk ic-1 and the Q-projection
of chunk ic+1 run as PE filler inside the units; the last V token-tiles run
as filler early in unit (0,0). Q/K quantization: DVE writes an fp8 staging
tile; two small SBUF->SBUF DMAs move the halves into the [64, 2, ...] layout
(lead-in: one per ring; main loop: both on the SP ring to keep the ACT
sequencer free for exp dispatch).
"""

import numpy as np

B, S, D = 4, 2048, 1280
HEADS = 10
HD = 128
NH = 5              # heads per core
P = 128
SCALE = float(D) ** -0.5
KT_D = D // P       # 10 k-tiles over D
KTP = KT_D // 2     # 5 DoubleRow k-tile pairs
NJT = S // P        # 16 j tiles
NIC = S // 512      # 4 i-chunks of 512
CX = 4.0            # x fp8 scale
CW = 64.0           # w fp8 scale
CQK = 1.0 / 32.0    # Q/K store rescale: psum 256x -> stored 8x true
EXP_SCALE = SCALE / 64.0   # dots psum carries (8*8)=64x true scale
CV = 1.0 / 256.0    # V store rescale: psum 256x -> true

_PROGRAM_CACHE = {}


def _build_program(repeat=1):
    if repeat in _PROGRAM_CACHE:
        return _PROGRAM_CACHE[repeat]

    import concourse.mybir as mybir
    from concourse import bacc
    import concourse.tile as tile

    F32 = mybir.dt.float32
    F32R = mybir.dt.float32r
    BF16 = mybir.dt.bfloat16
    F8 = mybir.dt.float8e4
    EXP = mybir.ActivationFunctionType.Exp
    DR = mybir.MatmulPerfMode.DoubleRow

    nc = bacc.Bacc()
    xh_d = nc.declare_dram_parameter("xh", [D, S], F8, isOutput=False)
    xl_d = nc.declare_dram_parameter("xl", [D, S], F8, isOutput=False)
    wqkh_d = nc.declare_dram_parameter("wqkh", [D, 2 * NH * HD], F8, isOutput=False)
    wvh_d = nc.declare_dram_parameter("wvh", [D, NH * HD], F8, isOutput=False)
    wvl_d = nc.declare_dram_parameter("wvl", [D, NH * HD], F8, isOutput=False)
    wout_d = nc.declare_dram_parameter("wout", [NH * HD, D], BF16, isOutput=False)
    onesr_d = nc.declare_dram_parameter("onesr_in", [1, P], F32, isOutput=False)
    out_d = nc.declare_dram_parameter("outT", [D, S], F32, isOutput=True)

    xh_t = xh_d[:].rearrange("(kt p) s -> p kt s", p=P)        # [128, 10, 2048]
    xl_t = xl_d[:].rearrange("(kt p) s -> p kt s", p=P)
    wqkh_t = wqkh_d[:].rearrange("(kt p) m -> p kt m", p=P)    # [128, 10, 1280]
    wvh_t = wvh_d[:].rearrange("(kt p) m -> p kt m", p=P)
    wvl_t = wvl_d[:].rearrange("(kt p) m -> p kt m", p=P)
    wout_t = wout_d[:].rearrange("(kt p) m -> p kt m", p=P)    # [128, 5, 1280]

    with tile.TileContext(nc) as tc:
        with (
            tc.tile_pool(name="persist", bufs=1) as persist,
            tc.tile_pool(name="oio", bufs=3) as oio,
            tc.tile_pool(name="work", bufs=4) as work,
            tc.tile_pool(name="ptp", bufs=6) as ptp,
            tc.tile_pool(name="work2", bufs=2) as work2,
            tc.tile_pool(name="stgp", bufs=6) as stgp,
            tc.tile_pool(name="ps_mm", bufs=2, space="PSUM") as ps_mm,
            tc.tile_pool(name="ps_acc", bufs=2, space="PSUM") as ps_acc,
            tc.tile_pool(name="ps_sm", bufs=2, space="PSUM") as ps_sm,
        ):
            XH = persist.tile([P, KT_D, S], F8, name="XH")
            XL = persist.tile([P, KT_D, S], F8, name="XL")
            WQKH = persist.tile([P, KT_D, 2 * NH * HD], F8, name="WQKH")
            WVH = persist.tile([P, KT_D, NH * HD], F8, name="WVH")
            WVL = persist.tile([P, KT_D, NH * HD], F8, name="WVL")
            WO = persist.tile([P, NH, D], BF16, name="WO")
            QS = persist.tile([64, 2, NH, S], F8, name="QS")
            KS = persist.tile([64, 2, NH, S], F8, name="KS")
            V = persist.tile([P, NJT, NH * HD], BF16, name="V")
            ones = persist.tile([P, P], BF16, name="ones")

            scr = persist.tile([P, 1], BF16, name="scr")
            nc.gpsimd.memset(ones[:], 1.0)
            # dummy exp: forces the Exp table load while ACT is idle, so the
            # first attention unit's exp doesn't pay the ~1.3us load
            nc.scalar.activation(scr[:], ones[:, 0:1], EXP, scale=1.0)

            def load_inputs():
                # Two HWDGE rings in parallel, each in consumption order.
                # SP ring: XH + K-column weights (the 1-term K projections'
                # whole critical path) + Q columns.  ACT ring: XL + V weights
                # + WO (V projections are the first consumer of XL).
                def w_m(m):
                    msl = slice(m * P, (m + 1) * P)
                    nc.sync.dma_start(WQKH[:, :, msl], wqkh_t[:, :, msl])

                def xh_ic(ic):
                    isl = slice(ic * 512, (ic + 1) * 512)
                    nc.sync.dma_start(XH[:, :, isl], xh_t[:, :, isl])

                def xl_ic(ic):
                    isl = slice(ic * 512, (ic + 1) * 512)
                    nc.scalar.dma_start(XL[:, :, isl], xl_t[:, :, isl])

                for m in range(2):
                    w_m(NH + m)
                xh_ic(0)
                nc.scalar.dma_start(WVH[:], wvh_t)
                for m in range(2, NH):
                    w_m(NH + m)
                xh_ic(1)
                nc.scalar.dma_start(WVL[:], wvl_t)
                xl_ic(0)
                xh_ic(2)
                xl_ic(1)
                xh_ic(3)
                xl_ic(2)
                for m in range(NH):
                    w_m(m)       # Q columns
                xl_ic(3)
                nc.scalar.dma_start(WO[:], wout_t)

            def proj_qk(m, ic, dst, h, main=False):
                """One [128,512] Q or K projection tile -> fp8 into dst[64,2,...].

                Q (dst is QS) runs 2 terms (xh+xl); K runs 1 term (xh only) -
                the e4m3 store noise dominates K's error either way."""
                isl = slice(ic * 512, (ic + 1) * 512)
                msl = slice(m * P, (m + 1) * P)
                terms = ((WQKH, msl, XH),)
                if dst is QS:
                    terms = ((WQKH, msl, XH), (WQKH, msl, XL))
                q_ps = ps_sm.tile([P, 512], F32, name="sm")
                nterm = len(terms) * KTP
                step = 0
                for ktp in range(KTP):
                    k2 = slice(2 * ktp, 2 * ktp + 2)
                    for Wt, wsl, Xt in terms:
                        nc.tensor.matmul(
                            q_ps[:], Wt[:, k2, wsl], Xt[:, k2, isl],
                            start=(step == 0), stop=(step == nterm - 1),
                            perf_mode=DR,
                        )
                        step += 1
                stg = stgp.tile([P, 512], F8, name="stg")
                nc.vector.tensor_scalar_mul(stg[:], q_ps[:], CQK)
                # layout-shift DMAs ride HWDGE rings only (SWDGE via the
                # Pool engine shows a ~10x slowdown red flag on real HW).
                # Main loop: both on the SP ring (its queue is light there)
                # so the ACT sequencer stays free for exp dispatch.
                lo_ring = nc.sync
                hi_ring = nc.sync if main else nc.scalar
                lo_ring.dma_start(dst[:, 0, h, isl], stg[0:64, :])
                hi_ring.dma_start(dst[:, 1, h, isl], stg[64:128, :])

            def proj_v(tt):
                """V rows for token-tile tt, direct-transposed: [128 tok, 640]."""
                tsl = slice(tt * P, (tt + 1) * P)
                v_ps = ps_mm.tile([P, 1024], F32, name="mm")
                step = 0
                for ktp in range(KTP):
                    k2 = slice(2 * ktp, 2 * ktp + 2)
                    for Xt, Wt in ((XH, WVH), (XH, WVL), (XL, WVH)):
                        st = (step == 0)
                        sp = (step == 3 * KTP - 1)
                        nc.tensor.matmul(v_ps[:, 0:512], Xt[:, k2, tsl],
                                         Wt[:, k2, 0:512], start=st, stop=sp,
                                         perf_mode=DR)
                        nc.tensor.matmul(v_ps[:, 512:640], Xt[:, k2, tsl],
                                         Wt[:, k2, 512:640], start=st, stop=sp,
                                         perf_mode=DR)
                        step += 1
                nc.vector.tensor_scalar_mul(V[:, tt], v_ps[:, 0:640], CV)

            for rep in range(repeat):
                if rep == 0:
                    load_inputs()

                # ---- lead-in: K chunks track the XH stream; V token-tiles
                # follow once XL + V weights land.  The last V tiles run as
                # PE filler inside unit (0,0) (consumed only at its last jp
                # iterations) ----
                for m in range(NH):
                    proj_qk(NH + m, 0, KS, m)
                for m in range(NH):
                    proj_qk(NH + m, 1, KS, m)
                for tt in range(0, 4):
                    proj_v(tt)
                for m in range(NH):
                    proj_qk(NH + m, 2, KS, m)
                for tt in range(4, 8):
                    proj_v(tt)
                for m in range(NH):
                    proj_qk(NH + m, 3, KS, m)
                for tt in range(8, 12):
                    proj_v(tt)
                for m in range(NH):
                    proj_qk(m, 0, QS, m)
                for tt in range(12, 14):
                    proj_v(tt)

                # ---- attention + out projection -----------------------------
                def norm_tail_a(st):
                    """fold halves -> one all-ones matmul = broadcast colsum."""
                    fold, o_ps, OT, h = st
                    fh = work2.tile([P, 512], BF16, name="fh", tag="fh")
                    nc.vector.tensor_add(fh[:], fold[:, :512], fold[:, 512:])
                    bc_ps = ps_sm.tile([P, 512], F32, name="sm")
                    nc.tensor.matmul(bc_ps[:], ones[:], fh[:],
                                     start=True, stop=True)
                    return (bc_ps, o_ps, OT, h)

                def norm_tail_b(st):
                    """reciprocal + normalize into OT (kept fp32)."""
                    bc_ps, o_ps, OT, h = st
                    rec = work2.tile([P, 512], F32, name="rec", tag="rec")
                    nc.vector.reciprocal(rec[:], bc_ps[:])
                    nc.vector.tensor_mul(OT[:, h, :], o_ps[:], rec[:])

                def norm_tail(st):
                    norm_tail_b(norm_tail_a(st))

                def out_proj(ic, OT, ms, final=False):
                    isl = slice(ic * 512, (ic + 1) * 512)
                    for i, m in enumerate(ms):
                        p_ps = ps_sm.tile([P, 512], F32, name="sm")
                        for kt in range(NH):
                            nc.tensor.matmul(
                                p_ps[:], WO[:, kt, m * P:(m + 1) * P],
                                OT[:, kt, :].bitcast(F32R),
                                start=(kt == 0), stop=(kt == NH - 1),
                            )
                        outc = work.tile([P, 512], F32, name="outc")
                        nc.vector.tensor_copy(outc[:], p_ps[:])
                        # spread the writes over both rings at the end so the
                        # final drain isn't serialized on one queue
                        ring = nc.scalar if (final and i % 2) else nc.sync
                        ring.dma_start(out_d[m * P:(m + 1) * P, isl], outc[:])

                pending_tail = None
                pending_proj = None
                tail_mid = None
                for ic in range(NIC):
                    isl = slice(ic * 512, (ic + 1) * 512)
                    OT = oio.tile([P, NH, 512], F32, name="OT")
                    for h in range(NH):
                        fold = work2.tile([P, 1024], BF16, name="fold", tag="fold")
                        o_ps = ps_acc.tile([P, 512], F32, name="acc")
                        pt2s = [None] * (NJT // 2)
                        # software-pipelined: the paired S-DR-matmuls + one
                        # wide exp run a pair ahead of the O-matmuls so PE
                        # never waits on ACT.
                        for jp in range(NJT // 2 + 2):
                            if jp < NJT // 2:
                                s_ps = ps_mm.tile([P, 1024], F32, name="mm")
                                for half in range(2):
                                    jt = 2 * jp + half
                                    jsl = slice(jt * P, (jt + 1) * P)
                                    nc.tensor.matmul(
                                        s_ps[:, half * 512:(half + 1) * 512],
                                        KS[:, :, h, jsl], QS[:, :, h, isl],
                                        start=True, stop=True, perf_mode=DR,
                                    )
                                pt2 = ptp.tile([P, 1024], BF16, name="pt")
                                nc.scalar.activation(pt2[:], s_ps[:], EXP,
                                                     scale=EXP_SCALE)
                                pt2s[jp] = pt2
                                if jp == 1:
                                    nc.vector.tensor_add(
                                        fold[:], pt2s[0][:], pt2s[1][:])
                                elif jp > 1:
                                    nc.vector.tensor_add(fold[:], fold[:], pt2[:])
                            if jp > 1:
                                prev = pt2s[jp - 2]
                                for half in range(2):
                                    jt = 2 * (jp - 2) + half
                                    nc.tensor.matmul(
                                        o_ps[:], V[:, jt, h * P:(h + 1) * P],
                                        prev[:, half * 512:(half + 1) * 512],
                                        start=(jt == 0), stop=(jt == NJT - 1),
                                    )
                            if ic == 0 and h == 0 and jp in (3, 5) and jp < NJT // 2:
                                # late V token-tiles as PE filler; consumed
                                # only at this unit's last jp iterations
                                proj_v(14 if jp == 3 else 15)
                            if jp == 0 and pending_tail is not None:
                                tail_mid = norm_tail_a(pending_tail)
                                pending_tail = None
                            if jp == 2 and tail_mid is not None:
                                norm_tail_b(tail_mid)
                                tail_mid = None
                            if jp in (4, 6) and pending_proj is not None:
                                pic, pOT = pending_proj
                                m0 = 2 * h + (0 if jp == 4 else 1)
                                out_proj(pic, pOT, [m0])
                                if h == NH - 1 and jp == 6:
                                    pending_proj = None
                            if jp == 5 and ic < NIC - 1:
                                # Q projection of the next chunk as PE filler
                                proj_qk(h, ic + 1, QS, h, main=True)
                        pending_tail = (fold, o_ps, OT, h)
                    pending_proj = (ic, OT)
                norm_tail(pending_tail)
                out_proj(*pending_proj, range(D // P), final=True)

    nc.finalize()
    _PROGRAM_CACHE[repeat] = nc
    return nc


def _enc_hi_lo(a, scale):
    """Split scale*a into e4m3 hi + lo (same scale; lo holds the residual)."""
    import ml_dtypes
    f8 = ml_dtypes.float8_e4m3
    sa = np.asarray(a, np.float32) * scale
    hi = sa.astype(f8)
    lo = (sa - hi.astype(np.float32)).astype(f8)
    return hi, lo


def _shard_inputs(x, w_qkv, w_out):
    """Build the 8 per-core input maps (fp8 hi/lo operands, host-encoded)."""
    import ml_dtypes
    bf16 = ml_dtypes.bfloat16
    onesr = np.ones((1, P), np.float32)
    in_maps = []
    for c in range(8):
        b = c // 2
        h0 = NH * (c % 2)
        xT = np.ascontiguousarray(np.asarray(x[b], np.float32).T)      # [D, S]
        xh, xl = _enc_hi_lo(xT, CX)
        qk = np.concatenate([
            w_qkv[:, qi * D + h0 * HD: qi * D + (h0 + NH) * HD] for qi in range(2)
        ], axis=1)                                                     # [D, 1280]
        wqkh, _ = _enc_hi_lo(qk, CW)   # Q 2-term / K 1-term: w plain fp8
        wv = w_qkv[:, 2 * D + h0 * HD: 2 * D + (h0 + NH) * HD]         # [D, 640]
        wvh, wvl = _enc_hi_lo(wv, CW)
        in_maps.append(dict(
            xh=xh, xl=xl, wqkh=wqkh,
            wvh=wvh, wvl=wvl,
            wout=np.ascontiguousarray(
                np.asarray(w_out[h0 * HD:(h0 + NH) * HD, :], np.float32)
            ).astype(bf16),
            onesr_in=onesr,
        ))
    return in_maps


def run_sharded(x, w_qkv, w_out, b_out, repeat=1, trace=False):
    """Run the SPMD program; returns (out [B,S,D], BassKernelResults)."""
    from concourse.bass_utils import run_bass_kernel_spmd

    nc = _build_program(repeat)
    in_maps = _shard_inputs(x, w_qkv, w_out)
    res = run_bass_kernel_spmd(nc, in_maps, list(range(8)), trace=trace)
    out = np.empty((B, S, D), np.float32)
    for b in range(B):
        out[b] = (res.results[2 * b]["outT"].T
                  + res.results[2 * b + 1]["outT"].T
                  + b_out[None, :])
    return out, res


def kernel(x, w_qkv, w_out, b_out):
    x = np.asarray(x, np.float32)
    w_qkv = np.asarray(w_qkv, np.float32)
    w_out = np.asarray(w_out, np.float32)
    b_out = np.asarray(b_out, np.float32)
    out, _ = run_sharded(x, w_qkv, w_out, b_out)
    return out


# revision 26
# speedup vs baseline: 1.0771x; 1.0363x over previous
"""Multi-head attention (B=4, S=2048, D=1280, H=10, hd=128) on 8 TRN2 NeuronCores.

Sharding: core c handles batch b = c//2 and heads h0 = 5*(c%2) .. h0+5
(data-parallel over batch x head-parallel tensor parallelism). Host does the
final pairwise all-reduce + bias.

Precision/speed scheme (PE is the bottleneck engine):
  - Projections run as fp8 DoubleRow matmuls (2 contraction k-tiles per pass,
    0.5 cyc/row) with *residual compensation*: operands split hi+lo in e4m3 at
    a fixed power-of-2 scale, cross terms accumulated in fp32 PSUM. V uses
    3 terms (x_hi*w_hi + x_lo*w_hi + x_hi*w_lo, ~bf16-grade); Q uses 2 terms
    (x compensated, w plain fp8); K uses 1 term (x_hi*w_hi only) - both are
    re-quantized to e4m3 for storage anyway, so the fp8 store noise dominates
    and the compensation terms beyond these don't move the end-to-end error.
  - Q,K are stored e4m3 (8x true scale) in the [64, 2, S] split-hd layout
    DoubleRow wants; S^T = K Q^T then runs fp8-DoubleRow at half bf16 cost.
  - P = exp(S*scale) stays bf16 (scale folds all fp8 scaling).  O = P V runs
    bf16; the normalized attention output OT is kept in fp32 and fed to the
    out-projection as an f32r moving operand (same 1 cyc/row as bf16 for
    free-dim >= 256), so the only out-projection noise is the bf16 W_out.
  - V is projected directly transposed (stationary x-tiles, moving w_v).
  - Softmax colsum: DVE folds P pairs to one [128,512] tile, then a single
    all-ones [128,128] stationary matmul produces the column sums already
    broadcast across all 128 partitions (one 512-row pass; no separate
    [1,512] sum + re-broadcast chain).

Schedule: x/weight DMAs split across the two HWDGE rings (SP: XH + K-column
weights + Q columns; ACT: XL + V weights + WO) in consumption order. K-proj
(1-term, XH-only) tracks the XH stream; V-proj token-tiles follow as both
streams land; Q chunk 0 last; then per (chunk, head) attention units with the
jp software pipeline. The out-projection of chunk ic-1 and the Q-projection
of chunk ic+1 run as PE filler inside the units; the last V token-tiles run
as filler early in unit (0,0). Q/K quantization: DVE writes an fp8 staging
tile; two small SBUF->SBUF DMAs move the halves into the [64, 2, ...] layout
(lead-in: one per ring; main loop: both on the SP ring to keep the ACT
sequencer free for exp dispatch).
"""

import numpy as np

B, S, D = 4, 2048, 1280
HEADS = 10
HD = 128
NH = 5              # heads per core
P = 128
SCALE = float(D) ** -0.5
KT_D = D // P       # 10 k-tiles over D
KTP = KT_D // 2     # 5 DoubleRow k-tile pairs
NJT = S // P        # 16 j tiles
NIC = S // 512      # 4 i-chunks of 512
CX = 4.0            # x fp8 scale
CW = 64.0           # w fp8 scale
CQK = 1.0 / 32.0    # Q/K store rescale: psum 256x -> stored 8x true
EXP_SCALE = SCALE / 64.0   # dots psum carries (8*8)=64x true scale
CV = 1.0 / 256.0    # V store rescale: psum 256x -> true

_PROGRAM_CACHE = {}


def _build_program(repeat=1):
    if repeat in _PROGRAM_CACHE:
        return _PROGRAM_CACHE[repeat]

    import concourse.mybir as mybir
    from concourse import bacc
    import concourse.tile as tile

    F32 = mybir.dt.float32
    F32R = mybir.dt.float32r
    BF16 = mybir.dt.bfloat16
    F8 = mybir.dt.float8e4
    EXP = mybir.ActivationFunctionType.Exp
    DR = mybir.MatmulPerfMode.DoubleRow

    nc = bacc.Bacc()
    xh_d = nc.declare_dram_parameter("xh", [D, S], F8, isOutput=False)
    xl_d = nc.declare_dram_parameter("xl", [D, S], F8, isOutput=False)
    wqkh_d = nc.declare_dram_parameter("wqkh", [D, 2 * NH * HD], F8, isOutput=False)
    wvh_d = nc.declare_dram_parameter("wvh", [D, NH * HD], F8, isOutput=False)
    wvl_d = nc.declare_dram_parameter("wvl", [D, NH * HD], F8, isOutput=False)
    wout_d = nc.declare_dram_parameter("wout", [NH * HD, D], BF16, isOutput=False)
    onesr_d = nc.declare_dram_parameter("onesr_in", [1, P], F32, isOutput=False)
    out_d = nc.declare_dram_parameter("outT", [D, S], F32, isOutput=True)

    # chunk-major x and m-major w layouts: each lead-in DMA writes one
    # contiguous free-range of its SBUF tile, and each matmul reads one, so
    # Tile's subtile dependency tracking stays exact (a chunk-0 read must not
    # serialize behind the chunk-3 DMA).
    xh_t = xh_d[:].rearrange("(kt p) (ic s) -> p ic kt s", p=P, ic=NIC)
    xl_t = xl_d[:].rearrange("(kt p) (ic s) -> p ic kt s", p=P, ic=NIC)
    wqkh_t = wqkh_d[:].rearrange("(kt p) (m c) -> p m kt c", p=P, c=P)
    wvh_t = wvh_d[:].rearrange("(kt p) m -> p kt m", p=P)
    wvl_t = wvl_d[:].rearrange("(kt p) m -> p kt m", p=P)
    wout_t = wout_d[:].rearrange("(kt p) m -> p kt m", p=P)    # [128, 5, 1280]

    with tile.TileContext(nc) as tc:
        with (
            tc.tile_pool(name="persist", bufs=1) as persist,
            tc.tile_pool(name="oio", bufs=3) as oio,
            tc.tile_pool(name="work", bufs=4) as work,
            tc.tile_pool(name="ptp", bufs=6) as ptp,
            tc.tile_pool(name="work2", bufs=2) as work2,
            tc.tile_pool(name="stgp", bufs=6) as stgp,
            tc.tile_pool(name="ps_mm", bufs=2, space="PSUM") as ps_mm,
            tc.tile_pool(name="ps_acc", bufs=2, space="PSUM") as ps_acc,
            tc.tile_pool(name="ps_sm", bufs=2, space="PSUM") as ps_sm,
        ):
            XH = persist.tile([P, NIC, KT_D, 512], F8, name="XH")
            XL = persist.tile([P, NIC, KT_D, 512], F8, name="XL")
            WQKH = persist.tile([P, 2 * NH, KT_D, P], F8, name="WQKH")
            WVH = persist.tile([P, KT_D, NH * HD], F8, name="WVH")
            WVL = persist.tile([P, KT_D, NH * HD], F8, name="WVL")
            WO = persist.tile([P, NH, D], BF16, name="WO")
            QS = persist.tile([64, 2, NH, S], F8, name="QS")
            KS = persist.tile([64, 2, NH, S], F8, name="KS")
            V = persist.tile([P, NJT, NH * HD], BF16, name="V")
            ones = persist.tile([P, P], BF16, name="ones")

            scr = persist.tile([P, 1], BF16, name="scr")
            nc.gpsimd.memset(ones[:], 1.0)
            # dummy exp: forces the Exp table load while ACT is idle, so the
            # first attention unit's exp doesn't pay the ~1.3us load
            nc.scalar.activation(scr[:], ones[:, 0:1], EXP, scale=1.0)

            def load_inputs():
                # All transfers serialize on the shared DMA engines, so the
                # issue order IS the arrival order.  Strict consumption order:
                # the first K weight column + XH0 unblock K(.,0); XL0 + V
                # weights next so V projections can start while the remaining
                # XH chunks stream for K(.,1..3).  The two rings only
                # parallelize dispatch.
                def w_m(m, ring):
                    ring.dma_start(WQKH[:, m], wqkh_t[:, m])

                def xh_ic(ic):
                    nc.sync.dma_start(XH[:, ic], xh_t[:, ic])

                def xl_ic(ic):
                    nc.scalar.dma_start(XL[:, ic], xl_t[:, ic])

                w_m(NH, nc.sync)
                xh_ic(0)
                xl_ic(0)
                w_m(NH + 1, nc.sync)
                nc.scalar.dma_start(WVH[:], wvh_t)
                w_m(NH + 2, nc.sync)
                nc.scalar.dma_start(WVL[:], wvl_t)
                w_m(NH + 3, nc.sync)
                w_m(NH + 4, nc.sync)
                xh_ic(1)
                xh_ic(2)
                xh_ic(3)
                xl_ic(1)
                xl_ic(2)
                xl_ic(3)
                for m in range(NH):
                    w_m(m, nc.scalar)     # Q columns
                # WO split per head: small transfers interleave benignly with
                # the latency-critical K/Q shift DMAs on the serialized DMA
                # engines (one 4.5us block would stall them).
                for kt in range(NH):
                    nc.scalar.dma_start(WO[:, kt], wout_t[:, kt])

            def proj_qk_mms(m, ic, dst, lo, hi, q_ps=None):
                """Emit DR matmul steps [lo, hi) of a Q/K projection.

                Q (dst is QS) runs 2 terms (xh+xl); K runs 1 term (xh only) -
                the e4m3 store noise dominates K's error either way.  Callers
                may split the step range across jp iterations to keep the
                per-jp PE load below the exp pace."""
                terms = (XH,) if dst is KS else (XH, XL)
                nterm = len(terms) * KTP
                if q_ps is None:
                    q_ps = ps_sm.tile([P, 512], F32, name="sm")
                for step in range(lo, hi):
                    ti, ktp = divmod(step, KTP)
                    k2 = slice(2 * ktp, 2 * ktp + 2)
                    nc.tensor.matmul(
                        q_ps[:], WQKH[:, m, k2], terms[ti][:, ic, k2],
                        start=(step == 0), stop=(step == nterm - 1),
                        perf_mode=DR,
                    )
                return q_ps

            def proj_qk_store(q_ps, ic, dst, h, main=False):
                isl = slice(ic * 512, (ic + 1) * 512)
                stg = stgp.tile([P, 512], F8, name="stg")
                nc.vector.tensor_scalar_mul(stg[:], q_ps[:], CQK)
                # layout-shift DMAs ride HWDGE rings only (SWDGE via the
                # Pool engine shows a ~10x slowdown red flag on real HW).
                # Main loop: both on the SP ring (its queue is light there)
                # so the ACT sequencer stays free for exp dispatch.
                lo_ring = nc.sync
                hi_ring = nc.sync if main else nc.scalar
                lo_ring.dma_start(dst[:, 0, h, isl], stg[0:64, :])
                hi_ring.dma_start(dst[:, 1, h, isl], stg[64:128, :])

            def proj_qk(m, ic, dst, h, main=False):
                nterm = (1 if dst is KS else 2) * KTP
                q_ps = proj_qk_mms(m, ic, dst, 0, nterm)
                proj_qk_store(q_ps, ic, dst, h, main=main)

            def proj_v(tt):
                """V rows for token-tile tt, direct-transposed: [128 tok, 640]."""
                ic, tsl = tt // 4, slice((tt % 4) * P, (tt % 4 + 1) * P)
                v_ps = ps_mm.tile([P, 1024], F32, name="mm")
                step = 0
                for ktp in range(KTP):
                    k2 = slice(2 * ktp, 2 * ktp + 2)
                    for Xt, Wt in ((XH, WVH), (XH, WVL), (XL, WVH)):
                        st = (step == 0)
                        sp = (step == 3 * KTP - 1)
                        nc.tensor.matmul(v_ps[:, 0:512], Xt[:, ic, k2, tsl],
                                         Wt[:, k2, 0:512], start=st, stop=sp,
                                         perf_mode=DR)
                        nc.tensor.matmul(v_ps[:, 512:640], Xt[:, ic, k2, tsl],
                                         Wt[:, k2, 512:640], start=st, stop=sp,
                                         perf_mode=DR)
                        step += 1
                nc.vector.tensor_scalar_mul(V[:, tt], v_ps[:, 0:640], CV)

            for rep in range(repeat):
                if rep == 0:
                    load_inputs()

                # ---- lead-in, paced by the serialized DMA stream: K(.,0)
                # tracks the early K-weight columns, the first V tiles slot in
                # as WVH/WVL/XL0 land, later K chunks track XH1..3, V follows
                # XL1..3.  The last V tiles run as PE filler inside unit (0,0)
                # (consumed only at its last jp iterations) ----
                for m in range(NH):
                    proj_qk(NH + m, 0, KS, m)
                for tt in range(0, 2):
                    proj_v(tt)
                for m in range(NH):
                    proj_qk(NH + m, 1, KS, m)
                for tt in range(2, 4):
                    proj_v(tt)
                for m in range(NH):
                    proj_qk(NH + m, 2, KS, m)
                for tt in range(4, 6):
                    proj_v(tt)
                for m in range(NH):
                    proj_qk(NH + m, 3, KS, m)
                for tt in range(6, 12):
                    proj_v(tt)
                for m in range(NH):
                    proj_qk(m, 0, QS, m)
                for tt in range(12, 14):
                    proj_v(tt)

                # ---- attention + out projection -----------------------------
                def norm_tail_a(st):
                    """fold halves -> one all-ones matmul = broadcast colsum."""
                    fold, o_ps, OT, h = st
                    fh = work2.tile([P, 512], BF16, name="fh", tag="fh")
                    nc.vector.tensor_add(fh[:], fold[:, :512], fold[:, 512:])
                    bc_ps = ps_sm.tile([P, 512], F32, name="sm")
                    nc.tensor.matmul(bc_ps[:], ones[:], fh[:],
                                     start=True, stop=True)
                    return (bc_ps, o_ps, OT, h)

                def norm_tail_b(st):
                    """reciprocal + normalize into OT (kept fp32)."""
                    bc_ps, o_ps, OT, h = st
                    rec = work2.tile([P, 512], F32, name="rec", tag="rec")
                    nc.vector.reciprocal(rec[:], bc_ps[:])
                    nc.vector.tensor_mul(OT[:, h, :], o_ps[:], rec[:])

                def norm_tail(st):
                    norm_tail_b(norm_tail_a(st))

                def op_mms(ic, OT, m, lo, hi, p_ps=None):
                    """Out-projection matmuls kt in [lo, hi) for column tile m.

                    Spread across jp iterations (2 matmuls each) so the per-jp
                    PE load stays below the exp pace."""
                    if p_ps is None:
                        p_ps = ps_sm.tile([P, 512], F32, name="sm")
                    for kt in range(lo, hi):
                        nc.tensor.matmul(
                            p_ps[:], WO[:, kt, m * P:(m + 1) * P],
                            OT[:, kt, :],
                            start=(kt == 0), stop=(kt == NH - 1),
                        )
                    return p_ps

                def op_store(p_ps, ic, m, final_i=None):
                    isl = slice(ic * 512, (ic + 1) * 512)
                    outc = work.tile([P, 512], F32, name="outc")
                    nc.vector.tensor_copy(outc[:], p_ps[:])
                    # spread the writes over both rings at the end so the
                    # final drain isn't serialized on one queue
                    ring = nc.scalar if (final_i is not None and final_i % 2) \
                        else nc.sync
                    ring.dma_start(out_d[m * P:(m + 1) * P, isl], outc[:])

                def out_proj(ic, OT, ms, final=False):
                    for i, m in enumerate(ms):
                        p_ps = op_mms(ic, OT, m, 0, NH)
                        op_store(p_ps, ic, m, final_i=i if final else None)

                # ---- attention units, software-pipelined ACROSS units: the
                # S matmuls + exp of unit u+1's first two j-pairs are emitted
                # during unit u's O-drain steps (jp8/jp9), so ACT never sees a
                # unit-boundary bubble.  Fillers (Q proj, out-proj) are spread
                # one-two matmuls per jp so the per-jp PE load stays under the
                # exp pace. ----
                units = [(ic, h) for ic in range(NIC) for h in range(NH)]
                ustate = {}

                def emit_S(u, sj):
                    """S-pair sj of unit u: 2 DR matmuls + exp + fold add."""
                    ic, h = units[u]
                    st = ustate[u]
                    isl = slice(ic * 512, (ic + 1) * 512)
                    s_ps = ps_mm.tile([P, 1024], F32, name="mm")
                    for half in range(2):
                        jt = 2 * sj + half
                        jsl = slice(jt * P, (jt + 1) * P)
                        nc.tensor.matmul(
                            s_ps[:, half * 512:(half + 1) * 512],
                            KS[:, :, h, jsl], QS[:, :, h, isl],
                            start=True, stop=True, perf_mode=DR,
                        )
                    pt2 = ptp.tile([P, 1024], BF16, name="pt")
                    nc.scalar.activation(pt2[:], s_ps[:], EXP, scale=EXP_SCALE)
                    st['pt2s'][sj] = pt2
                    if sj == 1:
                        nc.vector.tensor_add(
                            st['fold'][:], st['pt2s'][0][:], st['pt2s'][1][:])
                    elif sj > 1:
                        nc.vector.tensor_add(
                            st['fold'][:], st['fold'][:], pt2[:])

                def new_unit_state(u):
                    ic, h = units[u]
                    OT = (ustate[u - 1]['OT'] if h else
                          oio.tile([P, NH, 512], BF16, name="OT"))
                    ustate[u] = dict(
                        OT=OT,
                        fold=work2.tile([P, 1024], BF16, name="fold",
                                        tag="fold"),
                        pt2s=[None] * (NJT // 2),
                    )

                pending_tail = None
                pending_proj = None
                tail_mid = None
                new_unit_state(0)
                emit_S(0, 0)
                emit_S(0, 1)
                for u, (ic, h) in enumerate(units):
                    st = ustate[u]
                    o_ps = ps_acc.tile([P, 512], F32, name="acc")
                    for jp in range(NJT // 2 + 2):
                        if jp < NJT // 2 - 2:
                            emit_S(u, jp + 2)
                        elif jp >= NJT // 2 and u + 1 < len(units):
                            if jp == NJT // 2:
                                new_unit_state(u + 1)
                            emit_S(u + 1, jp - NJT // 2)
                        if jp > 1:
                            prev = st['pt2s'][jp - 2]
                            for half in range(2):
                                jt = 2 * (jp - 2) + half
                                nc.tensor.matmul(
                                    o_ps[:], V[:, jt, h * P:(h + 1) * P],
                                    prev[:, half * 512:(half + 1) * 512],
                                    start=(jt == 0), stop=(jt == NJT - 1),
                                )
                        if ic == 0 and h == 0 and jp in (3, 5):
                            # late V token-tiles as PE filler; consumed only
                            # at this unit's last jp iterations
                            proj_v(14 if jp == 3 else 15)
                        if jp == 0 and pending_tail is not None:
                            tail_mid = norm_tail_a(pending_tail)
                            pending_tail = None
                        if jp == 2 and tail_mid is not None:
                            norm_tail_b(tail_mid)
                            tail_mid = None
                        # Q projection of the next chunk, split jp0/jp1
                        if ic < NIC - 1:
                            if jp == 0:
                                qf_ps = proj_qk_mms(h, ic + 1, QS, 0, KTP)
                            elif jp == 1:
                                proj_qk_mms(h, ic + 1, QS, KTP, 2 * KTP,
                                            q_ps=qf_ps)
                                proj_qk_store(qf_ps, ic + 1, QS, h, main=True)
                        # out-projection of the previous chunk: 2 column
                        # tiles, one-two matmuls per jp over jp3..7
                        if pending_proj is not None and 3 <= jp <= 7:
                            pic, pOT = pending_proj
                            if jp == 3:
                                op_ps = op_mms(pic, pOT, 2 * h, 0, 2)
                            elif jp == 4:
                                op_mms(pic, pOT, 2 * h, 2, 4, p_ps=op_ps)
                            elif jp == 5:
                                op_mms(pic, pOT, 2 * h, 4, 5, p_ps=op_ps)
                                op_store(op_ps, pic, 2 * h)
                                op_ps = op_mms(pic, pOT, 2 * h + 1, 0, 1)
                            elif jp == 6:
                                op_mms(pic, pOT, 2 * h + 1, 1, 3, p_ps=op_ps)
                            elif jp == 7:
                                op_mms(pic, pOT, 2 * h + 1, 3, 5, p_ps=op_ps)
                                op_store(op_ps, pic, 2 * h + 1)
                                if h == NH - 1:
                                    pending_proj = None
                    pending_tail = (st['fold'], o_ps, st['OT'], h)
                    if h == NH - 1:
                        pending_proj = (ic, st['OT'])
                    if u > 0:
                        del ustate[u - 1]
                norm_tail(pending_tail)
                out_proj(*pending_proj, range(D // P), final=True)

    nc.finalize()
    _PROGRAM_CACHE[repeat] = nc
    return nc


def _enc_hi_lo(a, scale):
    """Split scale*a into e4m3 hi + lo (same scale; lo holds the residual)."""
    import ml_dtypes
    f8 = ml_dtypes.float8_e4m3
    sa = np.asarray(a, np.float32) * scale
    hi = sa.astype(f8)
    lo = (sa - hi.astype(np.float32)).astype(f8)
    return hi, lo


def _shard_inputs(x, w_qkv, w_out):
    """Build the 8 per-core input maps (fp8 hi/lo operands, host-encoded)."""
    import ml_dtypes
    bf16 = ml_dtypes.bfloat16
    onesr = np.ones((1, P), np.float32)
    in_maps = []
    for c in range(8):
        b = c // 2
        h0 = NH * (c % 2)
        xT = np.ascontiguousarray(np.asarray(x[b], np.float32).T)      # [D, S]
        xh, xl = _enc_hi_lo(xT, CX)
        qk = np.concatenate([
            w_qkv[:, qi * D + h0 * HD: qi * D + (h0 + NH) * HD] for qi in range(2)
        ], axis=1)                                                     # [D, 1280]
        wqkh, _ = _enc_hi_lo(qk, CW)   # Q 2-term / K 1-term: w plain fp8
        wv = w_qkv[:, 2 * D + h0 * HD: 2 * D + (h0 + NH) * HD]         # [D, 640]
        wvh, wvl = _enc_hi_lo(wv, CW)
        in_maps.append(dict(
            xh=xh, xl=xl, wqkh=wqkh,
            wvh=wvh, wvl=wvl,
            wout=np.ascontiguousarray(
                np.asarray(w_out[h0 * HD:(h0 + NH) * HD, :], np.float32)
            ).astype(bf16),
            onesr_in=onesr,
        ))
    return in_maps


def run_sharded(x, w_qkv, w_out, b_out, repeat=1, trace=False):
    """Run the SPMD program; returns (out [B,S,D], BassKernelResults)."""
    from concourse.bass_utils import run_bass_kernel_spmd

    nc = _build_program(repeat)
    in_maps = _shard_inputs(x, w_qkv, w_out)
    res = run_bass_kernel_spmd(nc, in_maps, list(range(8)), trace=trace)
    out = np.empty((B, S, D), np.float32)
    for b in range(B):
        out[b] = (res.results[2 * b]["outT"].T
                  + res.results[2 * b + 1]["outT"].T
                  + b_out[None, :])
    return out, res


def kernel(x, w_qkv, w_out, b_out):
    x = np.asarray(x, np.float32)
    w_qkv = np.asarray(w_qkv, np.float32)
    w_out = np.asarray(w_out, np.float32)
    b_out = np.asarray(b_out, np.float32)
    out, _ = run_sharded(x, w_qkv, w_out, b_out)
    return out


# revision 30
# speedup vs baseline: 1.0921x; 1.0140x over previous
"""Multi-head attention (B=4, S=2048, D=1280, H=10, hd=128) on 8 TRN2 NeuronCores.

Sharding: core c handles batch b = c//2 and heads h0 = 5*(c%2) .. h0+5
(data-parallel over batch x head-parallel tensor parallelism). Host does the
final pairwise all-reduce + bias.

Precision/speed scheme (PE is the bottleneck engine):
  - Projections run as fp8 DoubleRow matmuls (2 contraction k-tiles per pass,
    0.5 cyc/row) with *residual compensation*: operands split hi+lo in e4m3 at
    a fixed power-of-2 scale, cross terms accumulated in fp32 PSUM. V uses
    3 terms (x_hi*w_hi + x_lo*w_hi + x_hi*w_lo, ~bf16-grade); Q uses 2 terms
    (x compensated, w plain fp8); K uses 1 term (x_hi*w_hi only) - both are
    re-quantized to e4m3 for storage anyway, so the fp8 store noise dominates
    and the compensation terms beyond these don't move the end-to-end error.
  - Q,K are stored e4m3 (8x true scale) in the [64, 2, S] split-hd layout
    DoubleRow wants; S^T = K Q^T then runs fp8-DoubleRow at half bf16 cost.
  - P = exp(S*scale) stays bf16 (scale folds all fp8 scaling).  O = P V runs
    bf16; the normalized attention output OT is kept in fp32 and fed to the
    out-projection as an f32r moving operand (same 1 cyc/row as bf16 for
    free-dim >= 256), so the only out-projection noise is the bf16 W_out.
  - V is projected directly transposed (stationary x-tiles, moving w_v).
  - Softmax colsum: DVE folds P pairs to one [128,512] tile, then a single
    all-ones [128,128] stationary matmul produces the column sums already
    broadcast across all 128 partitions (one 512-row pass; no separate
    [1,512] sum + re-broadcast chain).

Schedule: x/weight DMAs split across the two HWDGE rings (SP: XH + K-column
weights + Q columns; ACT: XL + V weights + WO) in consumption order. K-proj
(1-term, XH-only) tracks the XH stream; V-proj token-tiles follow as both
streams land; Q chunk 0 last; then per (chunk, head) attention units with the
jp software pipeline. The out-projection of chunk ic-1 and the Q-projection
of chunk ic+1 run as PE filler inside the units; the last V token-tiles run
as filler early in unit (0,0). Q/K quantization: DVE writes an fp8 staging
tile; two small SBUF->SBUF DMAs move the halves into the [64, 2, ...] layout
(lead-in: one per ring; main loop: both on the SP ring to keep the ACT
sequencer free for exp dispatch).
"""

import numpy as np

B, S, D = 4, 2048, 1280
HEADS = 10
HD = 128
NH = 5              # heads per core
P = 128
SCALE = float(D) ** -0.5
KT_D = D // P       # 10 k-tiles over D
KTP = KT_D // 2     # 5 DoubleRow k-tile pairs
NJT = S // P        # 16 j tiles
NIC = S // 512      # 4 i-chunks of 512
CX = 4.0            # x fp8 scale
CW = 64.0           # w fp8 scale
CQK = 1.0 / 32.0    # Q/K store rescale: psum 256x -> stored 8x true
EXP_SCALE = SCALE / 64.0   # dots psum carries (8*8)=64x true scale
CV = 1.0 / 256.0    # V store rescale: psum 256x -> true

_PROGRAM_CACHE = {}


def _build_program(repeat=1):
    if repeat in _PROGRAM_CACHE:
        return _PROGRAM_CACHE[repeat]

    import concourse.mybir as mybir
    from concourse import bacc
    import concourse.tile as tile

    F32 = mybir.dt.float32
    F32R = mybir.dt.float32r
    BF16 = mybir.dt.bfloat16
    F8 = mybir.dt.float8e4
    EXP = mybir.ActivationFunctionType.Exp
    DR = mybir.MatmulPerfMode.DoubleRow

    nc = bacc.Bacc()
    xh_d = nc.declare_dram_parameter("xh", [D, S], F8, isOutput=False)
    xl_d = nc.declare_dram_parameter("xl", [D, S], F8, isOutput=False)
    wqkh_d = nc.declare_dram_parameter("wqkh", [D, 2 * NH * HD], F8, isOutput=False)
    wvh_d = nc.declare_dram_parameter("wvh", [D, NH * HD], F8, isOutput=False)
    wvl_d = nc.declare_dram_parameter("wvl", [D, NH * HD], F8, isOutput=False)
    wout_d = nc.declare_dram_parameter("wout", [NH * HD, D], BF16, isOutput=False)
    onesr_d = nc.declare_dram_parameter("onesr_in", [1, P], F32, isOutput=False)
    out_d = nc.declare_dram_parameter("outT", [D, S], F32, isOutput=True)

    # chunk-major x and m-major w layouts: each lead-in DMA writes one
    # contiguous free-range of its SBUF tile, and each matmul reads one, so
    # Tile's subtile dependency tracking stays exact (a chunk-0 read must not
    # serialize behind the chunk-3 DMA).
    xh_t = xh_d[:].rearrange("(kt p) (ic s) -> p ic kt s", p=P, ic=NIC)
    xl_t = xl_d[:].rearrange("(kt p) (ic s) -> p ic kt s", p=P, ic=NIC)
    wqkh_t = wqkh_d[:].rearrange("(kt p) (m c) -> p m kt c", p=P, c=P)
    wvh_t = wvh_d[:].rearrange("(kt p) m -> p kt m", p=P)
    wvl_t = wvl_d[:].rearrange("(kt p) m -> p kt m", p=P)
    wout_t = wout_d[:].rearrange("(kt p) m -> p kt m", p=P)    # [128, 5, 1280]

    with tile.TileContext(nc) as tc:
        with (
            tc.tile_pool(name="persist", bufs=1) as persist,
            tc.tile_pool(name="oio", bufs=3) as oio,
            tc.tile_pool(name="work", bufs=4) as work,
            tc.tile_pool(name="ptp", bufs=6) as ptp,
            tc.tile_pool(name="work2", bufs=2) as work2,
            tc.tile_pool(name="stgp", bufs=6) as stgp,
            tc.tile_pool(name="ps_mm", bufs=2, space="PSUM") as ps_mm,
            tc.tile_pool(name="ps_acc", bufs=2, space="PSUM") as ps_acc,
            tc.tile_pool(name="ps_sm", bufs=2, space="PSUM") as ps_sm,
        ):
            XH = persist.tile([P, NIC, KT_D, 512], F8, name="XH")
            XL = persist.tile([P, NIC, KT_D, 512], F8, name="XL")
            WQKH = persist.tile([P, 2 * NH, KT_D, P], F8, name="WQKH")
            WVH = persist.tile([P, KT_D, NH * HD], F8, name="WVH")
            WVL = persist.tile([P, KT_D, NH * HD], F8, name="WVL")
            WO = persist.tile([P, NH, D], BF16, name="WO")
            QS = persist.tile([64, 2, NH, S], F8, name="QS")
            KS = persist.tile([64, 2, NH, S], F8, name="KS")
            V = persist.tile([P, NJT, NH * HD], BF16, name="V")
            ones = persist.tile([P, P], BF16, name="ones")

            scr = persist.tile([P, 1], BF16, name="scr")
            nc.gpsimd.memset(ones[:], 1.0)
            # dummy exp: forces the Exp table load while ACT is idle, so the
            # first attention unit's exp doesn't pay the ~1.3us load
            nc.scalar.activation(scr[:], ones[:, 0:1], EXP, scale=1.0)

            def load_inputs():
                # All transfers serialize on the shared DMA engines, so the
                # issue order IS the arrival order.  Strict consumption order:
                # the first K weight column + XH0 unblock K(.,0); XL0 + V
                # weights next so V projections can start while the remaining
                # XH chunks stream for K(.,1..3).  The two rings only
                # parallelize dispatch.
                def w_m(m, ring):
                    ring.dma_start(WQKH[:, m], wqkh_t[:, m])

                def xh_ic(ic):
                    nc.sync.dma_start(XH[:, ic], xh_t[:, ic])

                def xl_ic(ic):
                    nc.scalar.dma_start(XL[:, ic], xl_t[:, ic])

                w_m(NH, nc.sync)
                xh_ic(0)
                xl_ic(0)
                w_m(NH + 1, nc.sync)
                w_m(NH + 2, nc.sync)
                w_m(NH + 3, nc.sync)
                w_m(NH + 4, nc.sync)
                nc.scalar.dma_start(WVH[:], wvh_t)
                nc.scalar.dma_start(WVL[:], wvl_t)
                xh_ic(1)
                xh_ic(2)
                xh_ic(3)
                xl_ic(1)
                for m in range(NH):
                    w_m(m, nc.scalar)     # Q columns
                xl_ic(2)
                xl_ic(3)
                # WO split per head: small transfers interleave benignly with
                # the latency-critical K/Q shift DMAs on the serialized DMA
                # engines (one 4.5us block would stall them).
                for kt in range(NH):
                    nc.scalar.dma_start(WO[:, kt], wout_t[:, kt])

            def proj_qk_mms(m, ic, dst, lo, hi, q_ps=None):
                """Emit DR matmul steps [lo, hi) of a Q/K projection.

                Q (dst is QS) runs 2 terms (xh+xl); K runs 1 term (xh only) -
                the e4m3 store noise dominates K's error either way.  Callers
                may split the step range across jp iterations to keep the
                per-jp PE load below the exp pace."""
                terms = (XH,) if dst is KS else (XH, XL)
                nterm = len(terms) * KTP
                if q_ps is None:
                    q_ps = ps_sm.tile([P, 512], F32, name="sm")
                for step in range(lo, hi):
                    ti, ktp = divmod(step, KTP)
                    k2 = slice(2 * ktp, 2 * ktp + 2)
                    nc.tensor.matmul(
                        q_ps[:], WQKH[:, m, k2], terms[ti][:, ic, k2],
                        start=(step == 0), stop=(step == nterm - 1),
                        perf_mode=DR,
                    )
                return q_ps

            def proj_qk_store(q_ps, ic, dst, h, main=False):
                isl = slice(ic * 512, (ic + 1) * 512)
                stg = stgp.tile([P, 512], F8, name="stg")
                nc.vector.tensor_scalar_mul(stg[:], q_ps[:], CQK)
                # ONE layout-shift DMA into the [64, 2, 512] DoubleRow split:
                # iteration order maps source partition 2p+two -> dst slot
                # (p, two), i.e. an even/odd interleave of hd rows.  Any
                # consistent hd relabeling cancels between K (stationary) and
                # Q (moving) in the S dot product, so this is exact - and it
                # halves the shift-DMA count on the serialized HWDGE stage
                # (~630ns fixed cost per DMA).  Rides HWDGE rings only (SWDGE
                # via Pool shows a ~10x slowdown red flag on real HW); in the
                # main loop it uses the SP ring so the ACT sequencer stays
                # free for exp dispatch.
                ring = nc.sync if main else nc.scalar
                ring.dma_start(dst[:, :, h, isl], stg[:])

            def proj_qk(m, ic, dst, h, main=False):
                nterm = (1 if dst is KS else 2) * KTP
                q_ps = proj_qk_mms(m, ic, dst, 0, nterm)
                proj_qk_store(q_ps, ic, dst, h, main=main)

            def proj_v(tt):
                """V rows for token-tile tt, direct-transposed: [128 tok, 640]."""
                ic, tsl = tt // 4, slice((tt % 4) * P, (tt % 4 + 1) * P)
                v_ps = ps_mm.tile([P, 1024], F32, name="mm")
                step = 0
                for ktp in range(KTP):
                    k2 = slice(2 * ktp, 2 * ktp + 2)
                    for Xt, Wt in ((XH, WVH), (XH, WVL), (XL, WVH)):
                        st = (step == 0)
                        sp = (step == 3 * KTP - 1)
                        nc.tensor.matmul(v_ps[:, 0:512], Xt[:, ic, k2, tsl],
                                         Wt[:, k2, 0:512], start=st, stop=sp,
                                         perf_mode=DR)
                        nc.tensor.matmul(v_ps[:, 512:640], Xt[:, ic, k2, tsl],
                                         Wt[:, k2, 512:640], start=st, stop=sp,
                                         perf_mode=DR)
                        step += 1
                nc.vector.tensor_scalar_mul(V[:, tt], v_ps[:, 0:640], CV)

            for rep in range(repeat):
                if rep == 0:
                    load_inputs()

                # ---- lead-in, paced by the serialized DMA stream: K(.,0)
                # tracks the early K-weight columns, the first V tiles slot in
                # as WVH/WVL/XL0 land, later K chunks track XH1..3, V follows
                # XL1..3.  The last V tiles run as PE filler inside unit (0,0)
                # (consumed only at its last jp iterations) ----
                for m in range(NH):
                    proj_qk(NH + m, 0, KS, m)
                for tt in range(0, 2):
                    proj_v(tt)
                for m in range(NH):
                    proj_qk(NH + m, 1, KS, m)
                for tt in range(2, 4):
                    proj_v(tt)
                for m in range(NH):
                    proj_qk(NH + m, 2, KS, m)
                for tt in range(4, 6):
                    proj_v(tt)
                for m in range(NH):
                    proj_qk(NH + m, 3, KS, m)
                for tt in range(6, 8):
                    proj_v(tt)
                for m in range(NH):
                    proj_qk(m, 0, QS, m)
                for tt in range(8, 14):
                    proj_v(tt)

                # ---- attention + out projection -----------------------------
                def norm_tail_a(st):
                    """fold halves -> one all-ones matmul = broadcast colsum."""
                    fold, o_ps, OT, h = st
                    fh = work2.tile([P, 512], BF16, name="fh", tag="fh")
                    nc.vector.tensor_add(fh[:], fold[:, :512], fold[:, 512:])
                    bc_ps = ps_sm.tile([P, 512], F32, name="sm")
                    nc.tensor.matmul(bc_ps[:], ones[:], fh[:],
                                     start=True, stop=True)
                    return (bc_ps, o_ps, OT, h)

                def norm_tail_b(st):
                    """reciprocal + normalize into OT (kept fp32)."""
                    bc_ps, o_ps, OT, h = st
                    rec = work2.tile([P, 512], F32, name="rec", tag="rec")
                    nc.vector.reciprocal(rec[:], bc_ps[:])
                    nc.vector.tensor_mul(OT[:, h, :], o_ps[:], rec[:])

                def norm_tail(st):
                    norm_tail_b(norm_tail_a(st))

                def op_mms(ic, OT, m, lo, hi, p_ps=None):
                    """Out-projection matmuls kt in [lo, hi) for column tile m.

                    Spread across jp iterations (2 matmuls each) so the per-jp
                    PE load stays below the exp pace."""
                    if p_ps is None:
                        p_ps = ps_sm.tile([P, 512], F32, name="sm")
                    for kt in range(lo, hi):
                        nc.tensor.matmul(
                            p_ps[:], WO[:, kt, m * P:(m + 1) * P],
                            OT[:, kt, :],
                            start=(kt == 0), stop=(kt == NH - 1),
                        )
                    return p_ps

                def op_store(p_ps, ic, m, final_i=None):
                    isl = slice(ic * 512, (ic + 1) * 512)
                    outc = work.tile([P, 512], F32, name="outc")
                    nc.vector.tensor_copy(outc[:], p_ps[:])
                    # spread the writes over both rings at the end so the
                    # final drain isn't serialized on one queue
                    ring = nc.scalar if (final_i is not None and final_i % 2) \
                        else nc.sync
                    ring.dma_start(out_d[m * P:(m + 1) * P, isl], outc[:])

                def out_proj(ic, OT, ms, final=False):
                    for i, m in enumerate(ms):
                        p_ps = op_mms(ic, OT, m, 0, NH)
                        op_store(p_ps, ic, m, final_i=i if final else None)

                # ---- attention units, software-pipelined ACROSS units: the
                # S matmuls + exp of unit u+1's first two j-pairs are emitted
                # during unit u's O-drain steps (jp8/jp9), so ACT never sees a
                # unit-boundary bubble.  Fillers (Q proj, out-proj) are spread
                # one-two matmuls per jp so the per-jp PE load stays under the
                # exp pace. ----
                units = [(ic, h) for ic in range(NIC) for h in range(NH)]
                ustate = {}

                def emit_S(u, sj):
                    """S-pair sj of unit u: 2 DR matmuls + exp + fold add."""
                    ic, h = units[u]
                    st = ustate[u]
                    isl = slice(ic * 512, (ic + 1) * 512)
                    s_ps = ps_mm.tile([P, 1024], F32, name="mm")
                    for half in range(2):
                        jt = 2 * sj + half
                        jsl = slice(jt * P, (jt + 1) * P)
                        nc.tensor.matmul(
                            s_ps[:, half * 512:(half + 1) * 512],
                            KS[:, :, h, jsl], QS[:, :, h, isl],
                            start=True, stop=True, perf_mode=DR,
                        )
                    pt2 = ptp.tile([P, 1024], BF16, name="pt")
                    nc.scalar.activation(pt2[:], s_ps[:], EXP, scale=EXP_SCALE)
                    st['pt2s'][sj] = pt2
                    if sj == 1:
                        nc.vector.tensor_add(
                            st['fold'][:], st['pt2s'][0][:], st['pt2s'][1][:])
                    elif sj > 1:
                        nc.vector.tensor_add(
                            st['fold'][:], st['fold'][:], pt2[:])

                def new_unit_state(u):
                    ic, h = units[u]
                    OT = (ustate[u - 1]['OT'] if h else
                          oio.tile([P, NH, 512], BF16, name="OT"))
                    ustate[u] = dict(
                        OT=OT,
                        fold=work2.tile([P, 1024], BF16, name="fold",
                                        tag="fold"),
                        pt2s=[None] * (NJT // 2),
                    )

                pending_tail = None
                pending_proj = None
                tail_mid = None
                new_unit_state(0)
                emit_S(0, 0)
                emit_S(0, 1)
                for u, (ic, h) in enumerate(units):
                    st = ustate[u]
                    o_ps = ps_acc.tile([P, 512], F32, name="acc")
                    for jp in range(NJT // 2 + 2):
                        if jp < NJT // 2 - 2:
                            emit_S(u, jp + 2)
                        elif jp >= NJT // 2 and u + 1 < len(units):
                            if jp == NJT // 2:
                                new_unit_state(u + 1)
                            emit_S(u + 1, jp - NJT // 2)
                        if jp > 1:
                            prev = st['pt2s'][jp - 2]
                            for half in range(2):
                                jt = 2 * (jp - 2) + half
                                nc.tensor.matmul(
                                    o_ps[:], V[:, jt, h * P:(h + 1) * P],
                                    prev[:, half * 512:(half + 1) * 512],
                                    start=(jt == 0), stop=(jt == NJT - 1),
                                )
                        if ic == 0 and h == 0 and jp in (3, 5):
                            # late V token-tiles as PE filler; consumed only
                            # at this unit's last jp iterations
                            proj_v(14 if jp == 3 else 15)
                        if jp == 0 and pending_tail is not None:
                            tail_mid = norm_tail_a(pending_tail)
                            pending_tail = None
                        if jp == 2 and tail_mid is not None:
                            norm_tail_b(tail_mid)
                            tail_mid = None
                        # Q projection of the next chunk, split jp0/jp1
                        if ic < NIC - 1:
                            if jp == 0:
                                qf_ps = proj_qk_mms(h, ic + 1, QS, 0, KTP)
                            elif jp == 1:
                                proj_qk_mms(h, ic + 1, QS, KTP, 2 * KTP,
                                            q_ps=qf_ps)
                                proj_qk_store(qf_ps, ic + 1, QS, h, main=True)
                        # out-projection of the previous chunk: 2 column
                        # tiles, one-two matmuls per jp over jp3..7
                        if pending_proj is not None and 3 <= jp <= 7:
                            pic, pOT = pending_proj
                            if jp == 3:
                                op_ps = op_mms(pic, pOT, 2 * h, 0, 2)
                            elif jp == 4:
                                op_mms(pic, pOT, 2 * h, 2, 4, p_ps=op_ps)
                            elif jp == 5:
                                op_mms(pic, pOT, 2 * h, 4, 5, p_ps=op_ps)
                                op_store(op_ps, pic, 2 * h)
                                op_ps = op_mms(pic, pOT, 2 * h + 1, 0, 1)
                            elif jp == 6:
                                op_mms(pic, pOT, 2 * h + 1, 1, 3, p_ps=op_ps)
                            elif jp == 7:
                                op_mms(pic, pOT, 2 * h + 1, 3, 5, p_ps=op_ps)
                                op_store(op_ps, pic, 2 * h + 1)
                                if h == NH - 1:
                                    pending_proj = None
                    pending_tail = (st['fold'], o_ps, st['OT'], h)
                    if h == NH - 1:
                        pending_proj = (ic, st['OT'])
                    if u > 0:
                        del ustate[u - 1]
                norm_tail(pending_tail)
                out_proj(*pending_proj, range(D // P), final=True)

    nc.finalize()
    _PROGRAM_CACHE[repeat] = nc
    return nc


def _enc_hi_lo(a, scale):
    """Split scale*a into e4m3 hi + lo (same scale; lo holds the residual)."""
    import ml_dtypes
    f8 = ml_dtypes.float8_e4m3
    sa = np.asarray(a, np.float32) * scale
    hi = sa.astype(f8)
    lo = (sa - hi.astype(np.float32)).astype(f8)
    return hi, lo


def _shard_inputs(x, w_qkv, w_out):
    """Build the 8 per-core input maps (fp8 hi/lo operands, host-encoded)."""
    import ml_dtypes
    bf16 = ml_dtypes.bfloat16
    onesr = np.ones((1, P), np.float32)
    in_maps = []
    for c in range(8):
        b = c // 2
        h0 = NH * (c % 2)
        xT = np.ascontiguousarray(np.asarray(x[b], np.float32).T)      # [D, S]
        xh, xl = _enc_hi_lo(xT, CX)
        qk = np.concatenate([
            w_qkv[:, qi * D + h0 * HD: qi * D + (h0 + NH) * HD] for qi in range(2)
        ], axis=1)                                                     # [D, 1280]
        wqkh, _ = _enc_hi_lo(qk, CW)   # Q 2-term / K 1-term: w plain fp8
        wv = w_qkv[:, 2 * D + h0 * HD: 2 * D + (h0 + NH) * HD]         # [D, 640]
        wvh, wvl = _enc_hi_lo(wv, CW)
        in_maps.append(dict(
            xh=xh, xl=xl, wqkh=wqkh,
            wvh=wvh, wvl=wvl,
            wout=np.ascontiguousarray(
                np.asarray(w_out[h0 * HD:(h0 + NH) * HD, :], np.float32)
            ).astype(bf16),
            onesr_in=onesr,
        ))
    return in_maps


def run_sharded(x, w_qkv, w_out, b_out, repeat=1, trace=False):
    """Run the SPMD program; returns (out [B,S,D], BassKernelResults)."""
    from concourse.bass_utils import run_bass_kernel_spmd

    nc = _build_program(repeat)
    in_maps = _shard_inputs(x, w_qkv, w_out)
    res = run_bass_kernel_spmd(nc, in_maps, list(range(8)), trace=trace)
    out = np.empty((B, S, D), np.float32)
    for b in range(B):
        out[b] = (res.results[2 * b]["outT"].T
                  + res.results[2 * b + 1]["outT"].T
                  + b_out[None, :])
    return out, res


def kernel(x, w_qkv, w_out, b_out):
    x = np.asarray(x, np.float32)
    w_qkv = np.asarray(w_qkv, np.float32)
    w_out = np.asarray(w_out, np.float32)
    b_out = np.asarray(b_out, np.float32)
    out, _ = run_sharded(x, w_qkv, w_out, b_out)
    return out


# revision 40
# speedup vs baseline: 1.1050x; 1.0118x over previous
"""Multi-head attention (B=4, S=2048, D=1280, H=10, hd=128) on 8 TRN2 NeuronCores.

Sharding: core c handles batch b = c//2 and heads h0 = 5*(c%2) .. h0+5
(data-parallel over batch x head-parallel tensor parallelism). Host does the
final pairwise all-reduce + bias.

Precision/speed scheme (PE is the bottleneck engine):
  - Projections run as fp8 DoubleRow matmuls (2 contraction k-tiles per pass,
    0.5 cyc/row) with *residual compensation*: operands split hi+lo in e4m3 at
    a fixed power-of-2 scale, cross terms accumulated in fp32 PSUM. V uses
    3 terms (x_hi*w_hi + x_lo*w_hi + x_hi*w_lo, ~bf16-grade); Q uses 2 terms
    (x compensated, w plain fp8); K uses 1 term (x_hi*w_hi only) - both are
    re-quantized to e4m3 for storage anyway, so the fp8 store noise dominates
    and the compensation terms beyond these don't move the end-to-end error.
  - Q,K are stored e4m3 (8x true scale) in the [64, 2, S] split-hd layout
    DoubleRow wants; S^T = K Q^T then runs fp8-DoubleRow at half bf16 cost.
  - P = exp(S*scale) stays bf16 (scale folds all fp8 scaling); O = P V and
    the out-projection run bf16 (mixing f32r with bf16 matmul operands is
    rejected by the neuron compiler, so OT cannot ride fp32 for free).
  - V is projected directly transposed (stationary x-tiles, moving w_v).
  - Softmax colsum: DVE folds P pairs to one [128,512] tile, then a single
    all-ones [128,128] stationary matmul produces the column sums already
    broadcast across all 128 partitions (one 512-row pass; no separate
    [1,512] sum + re-broadcast chain).

Schedule (the cost model serializes every DMA on one shared DMA-engine pool
and charges a fixed ~630ns HWDGE slot per DMA, so DMA COUNT and arrival
order dominate the lead-in): inputs stream in strict consumption order -
first K weight column, XH0, XL0, remaining K columns, V weights, XH1..3,
XL1..3 with Q columns between, WO last and split per head so no bulk
transfer blocks a latency-critical shift. K-proj (1-term, XH-only) tracks
the XH stream with V-proj token-tiles slotted in; Q chunk 0 closes the
lead-in; the last V token-tiles run as PE filler inside unit (0,0).

Attention units are software-pipelined ACROSS units: unit u+1's first two
S-pairs + exps are emitted during unit u's O-drain steps (jp8/jp9), so ACT
(the unit-level bottleneck at 8x1038ns of exp per unit) sees no boundary
bubble.  PE fillers are spread so no jp exceeds the exp pace: the
out-projection of chunk ic-1 runs one-two matmuls per jp over jp3..7, and
the Q-projection of chunk ic+1 rides the exp-gated drain steps jp8/jp9.
Q/K quantization: DVE writes an fp8 staging tile; ONE SBUF->SBUF DMA moves
it into the [64, 2, ...] DoubleRow layout via an even/odd partition
interleave (any consistent hd relabeling cancels between K-stationary and
Q-moving in the S dot product).
"""

import numpy as np

B, S, D = 4, 2048, 1280
HEADS = 10
HD = 128
NH = 5              # heads per core
P = 128
SCALE = float(D) ** -0.5
KT_D = D // P       # 10 k-tiles over D
KTP = KT_D // 2     # 5 DoubleRow k-tile pairs
NJT = S // P        # 16 j tiles
NIC = S // 512      # 4 i-chunks of 512
CX = 4.0            # x fp8 scale
CW = 64.0           # w fp8 scale
CQK = 1.0 / 32.0    # Q/K store rescale: psum 256x -> stored 8x true
EXP_SCALE = SCALE / 64.0   # dots psum carries (8*8)=64x true scale
CV = 1.0 / 256.0    # V store rescale: psum 256x -> true

_PROGRAM_CACHE = {}


def _build_program(repeat=1):
    if repeat in _PROGRAM_CACHE:
        return _PROGRAM_CACHE[repeat]

    import concourse.mybir as mybir
    from concourse import bacc
    import concourse.tile as tile

    F32 = mybir.dt.float32
    F32R = mybir.dt.float32r
    BF16 = mybir.dt.bfloat16
    F8 = mybir.dt.float8e4
    EXP = mybir.ActivationFunctionType.Exp
    DR = mybir.MatmulPerfMode.DoubleRow

    nc = bacc.Bacc()
    xh_d = nc.declare_dram_parameter("xh", [D, S], F8, isOutput=False)
    xl_d = nc.declare_dram_parameter("xl", [D, S], F8, isOutput=False)
    wqkh_d = nc.declare_dram_parameter("wqkh", [D, 2 * NH * HD], F8, isOutput=False)
    wvh_d = nc.declare_dram_parameter("wvh", [D, NH * HD], F8, isOutput=False)
    wvl_d = nc.declare_dram_parameter("wvl", [D, NH * HD], F8, isOutput=False)
    wout_d = nc.declare_dram_parameter("wout", [NH * HD, D], BF16, isOutput=False)
    onesr_d = nc.declare_dram_parameter("onesr_in", [1, P], F32, isOutput=False)
    out_d = nc.declare_dram_parameter("outT", [D, S], F32, isOutput=True)

    # chunk-major x and m-major w layouts: each lead-in DMA writes one
    # contiguous free-range of its SBUF tile, and each matmul reads one, so
    # Tile's subtile dependency tracking stays exact (a chunk-0 read must not
    # serialize behind the chunk-3 DMA).
    xh_t = xh_d[:].rearrange("(kt p) (ic s) -> p ic kt s", p=P, ic=NIC)
    xl_t = xl_d[:].rearrange("(kt p) (ic s) -> p ic kt s", p=P, ic=NIC)
    wqkh_t = wqkh_d[:].rearrange("(kt p) (m c) -> p m kt c", p=P, c=P)
    wvh_t = wvh_d[:].rearrange("(kt p) m -> p kt m", p=P)
    wvl_t = wvl_d[:].rearrange("(kt p) m -> p kt m", p=P)
    wout_t = wout_d[:].rearrange("(kt p) m -> p kt m", p=P)    # [128, 5, 1280]

    with tile.TileContext(nc) as tc:
        with (
            tc.tile_pool(name="persist", bufs=1) as persist,
            tc.tile_pool(name="oio", bufs=3) as oio,
            tc.tile_pool(name="work", bufs=4) as work,
            tc.tile_pool(name="ptp", bufs=8) as ptp,
            tc.tile_pool(name="work2", bufs=2) as work2,
            tc.tile_pool(name="stgp", bufs=6) as stgp,
            tc.tile_pool(name="ps_mm", bufs=2, space="PSUM") as ps_mm,
            tc.tile_pool(name="ps_acc", bufs=2, space="PSUM") as ps_acc,
            tc.tile_pool(name="ps_sm", bufs=2, space="PSUM") as ps_sm,
        ):
            XH = persist.tile([P, NIC, KT_D, 512], F8, name="XH")
            XL = persist.tile([P, NIC, KT_D, 512], F8, name="XL")
            WQKH = persist.tile([P, 2 * NH, KT_D, P], F8, name="WQKH")
            WVH = persist.tile([P, KT_D, NH * HD], F8, name="WVH")
            WVL = persist.tile([P, KT_D, NH * HD], F8, name="WVL")
            WO = persist.tile([P, NH, D], BF16, name="WO")
            QS = persist.tile([64, 2, NH, S], F8, name="QS")
            KS = persist.tile([64, 2, NH, S], F8, name="KS")
            V = persist.tile([P, NJT, NH * HD], BF16, name="V")
            ones = persist.tile([P, P], BF16, name="ones")

            scr = persist.tile([P, 1], BF16, name="scr")
            nc.gpsimd.memset(ones[:], 1.0)
            # dummy exp: forces the Exp table load while ACT is idle, so the
            # first attention unit's exp doesn't pay the ~1.3us load
            nc.scalar.activation(scr[:], ones[:, 0:1], EXP, scale=1.0)

            def load_inputs():
                # All transfers serialize on the shared DMA engines, so the
                # issue order IS the arrival order.  Strict consumption order:
                # the first K weight column + XH0 unblock K(.,0); XL0 + V
                # weights next so V projections can start while the remaining
                # XH chunks stream for K(.,1..3).  The two rings only
                # parallelize dispatch.
                def w_m(m, ring):
                    ring.dma_start(WQKH[:, m], wqkh_t[:, m])

                def xh_ic(ic):
                    nc.sync.dma_start(XH[:, ic], xh_t[:, ic])

                def xl_ic(ic):
                    nc.scalar.dma_start(XL[:, ic], xl_t[:, ic])

                w_m(NH, nc.sync)
                xh_ic(0)
                xl_ic(0)
                w_m(NH + 1, nc.sync)
                w_m(NH + 2, nc.sync)
                w_m(NH + 3, nc.sync)
                w_m(NH + 4, nc.sync)
                nc.scalar.dma_start(WVH[:], wvh_t)
                nc.scalar.dma_start(WVL[:], wvl_t)
                xh_ic(1)
                xh_ic(2)
                xh_ic(3)
                xl_ic(1)
                for m in range(NH):
                    w_m(m, nc.scalar)     # Q columns
                xl_ic(2)
                xl_ic(3)
                # WO split per head: small transfers interleave benignly with
                # the latency-critical K/Q shift DMAs on the serialized DMA
                # engines (one 4.5us block would stall them).
                for kt in range(NH):
                    nc.scalar.dma_start(WO[:, kt], wout_t[:, kt])

            def proj_qk_mms(m, ic, dst, lo, hi, q_ps=None):
                """Emit DR matmul steps [lo, hi) of a Q/K projection.

                Q (dst is QS) runs 2 terms (xh+xl); K runs 1 term (xh only) -
                the e4m3 store noise dominates K's error either way.  Callers
                may split the step range across jp iterations to keep the
                per-jp PE load below the exp pace."""
                terms = (XH,) if dst is KS else (XH, XL)
                nterm = len(terms) * KTP
                if q_ps is None:
                    q_ps = ps_sm.tile([P, 512], F32, name="sm")
                for step in range(lo, hi):
                    ti, ktp = divmod(step, KTP)
                    k2 = slice(2 * ktp, 2 * ktp + 2)
                    nc.tensor.matmul(
                        q_ps[:], WQKH[:, m, k2], terms[ti][:, ic, k2],
                        start=(step == 0), stop=(step == nterm - 1),
                        perf_mode=DR,
                    )
                return q_ps

            def proj_qk_store(q_ps, ic, dst, h, main=False):
                isl = slice(ic * 512, (ic + 1) * 512)
                stg = stgp.tile([P, 512], F8, name="stg")
                nc.vector.tensor_scalar_mul(stg[:], q_ps[:], CQK)
                # ONE layout-shift DMA into the [64, 2, 512] DoubleRow split:
                # iteration order maps source partition 2p+two -> dst slot
                # (p, two), i.e. an even/odd interleave of hd rows.  Any
                # consistent hd relabeling cancels between K (stationary) and
                # Q (moving) in the S dot product, so this is exact - and it
                # halves the shift-DMA count on the serialized HWDGE stage
                # (~630ns fixed cost per DMA).  Rides HWDGE rings only (SWDGE
                # via Pool shows a ~10x slowdown red flag on real HW); in the
                # main loop it uses the SP ring so the ACT sequencer stays
                # free for exp dispatch.
                ring = nc.sync if main else nc.scalar
                ring.dma_start(dst[:, :, h, isl], stg[:])

            def proj_qk(m, ic, dst, h, main=False):
                nterm = (1 if dst is KS else 2) * KTP
                q_ps = proj_qk_mms(m, ic, dst, 0, nterm)
                proj_qk_store(q_ps, ic, dst, h, main=main)

            def proj_v(tt):
                """V rows for token-tile tt, direct-transposed: [128 tok, 640]."""
                ic, tsl = tt // 4, slice((tt % 4) * P, (tt % 4 + 1) * P)
                v_ps = ps_mm.tile([P, 1024], F32, name="mm")
                step = 0
                for ktp in range(KTP):
                    k2 = slice(2 * ktp, 2 * ktp + 2)
                    for Xt, Wt in ((XH, WVH), (XH, WVL), (XL, WVH)):
                        st = (step == 0)
                        sp = (step == 3 * KTP - 1)
                        nc.tensor.matmul(v_ps[:, 0:512], Xt[:, ic, k2, tsl],
                                         Wt[:, k2, 0:512], start=st, stop=sp,
                                         perf_mode=DR)
                        nc.tensor.matmul(v_ps[:, 512:640], Xt[:, ic, k2, tsl],
                                         Wt[:, k2, 512:640], start=st, stop=sp,
                                         perf_mode=DR)
                        step += 1
                nc.vector.tensor_scalar_mul(V[:, tt], v_ps[:, 0:640], CV)

            for rep in range(repeat):
                if rep == 0:
                    load_inputs()

                # ---- lead-in, paced by the serialized DMA stream: K(.,0)
                # tracks the early K-weight columns, the first V tiles slot in
                # as WVH/WVL/XL0 land, later K chunks track XH1..3, V follows
                # XL1..3.  The last V tiles run as PE filler inside unit (0,0)
                # (consumed only at its last jp iterations) ----
                for m in range(NH):
                    proj_qk(NH + m, 0, KS, m)
                for tt in range(0, 2):
                    proj_v(tt)
                for m in range(NH):
                    proj_qk(NH + m, 1, KS, m)
                for tt in range(2, 4):
                    proj_v(tt)
                for m in range(NH):
                    proj_qk(NH + m, 2, KS, m)
                for tt in range(4, 6):
                    proj_v(tt)
                for m in range(NH):
                    proj_qk(NH + m, 3, KS, m)
                for tt in range(6, 8):
                    proj_v(tt)
                for m in range(NH):
                    proj_qk(m, 0, QS, m)
                for tt in range(8, 14):
                    proj_v(tt)

                # ---- attention + out projection -----------------------------
                def norm_tail_a(st):
                    """fold halves -> one all-ones matmul = broadcast colsum."""
                    fold, o_ps, OT, h = st
                    fh = work2.tile([P, 512], BF16, name="fh", tag="fh")
                    nc.vector.tensor_add(fh[:], fold[:, :512], fold[:, 512:])
                    bc_ps = ps_sm.tile([P, 512], F32, name="sm")
                    nc.tensor.matmul(bc_ps[:], ones[:], fh[:],
                                     start=True, stop=True)
                    return (bc_ps, o_ps, OT, h)

                def norm_tail_b(st):
                    """reciprocal + normalize into OT (kept fp32)."""
                    bc_ps, o_ps, OT, h = st
                    rec = work2.tile([P, 512], F32, name="rec", tag="rec")
                    nc.vector.reciprocal(rec[:], bc_ps[:])
                    nc.vector.tensor_mul(OT[:, h, :], o_ps[:], rec[:])

                def norm_tail(st):
                    norm_tail_b(norm_tail_a(st))

                def op_mms(ic, OT, m, lo, hi, p_ps=None):
                    """Out-projection matmuls kt in [lo, hi) for column tile m.

                    Spread across jp iterations (2 matmuls each) so the per-jp
                    PE load stays below the exp pace."""
                    if p_ps is None:
                        p_ps = ps_sm.tile([P, 512], F32, name="sm")
                    for kt in range(lo, hi):
                        nc.tensor.matmul(
                            p_ps[:], WO[:, kt, m * P:(m + 1) * P],
                            OT[:, kt, :],
                            start=(kt == 0), stop=(kt == NH - 1),
                        )
                    return p_ps

                def op_store(p_ps, ic, m, final_i=None, pair=None):
                    isl = slice(ic * 512, (ic + 1) * 512)
                    if pair is not None:
                        # copy halves into a [128,1024] tile; one DMA per m
                        # pair (the serialized HWDGE stage costs ~630ns per
                        # DMA regardless of size)
                        outc2, half = pair
                        nc.vector.tensor_copy(
                            outc2[:, half * 512:(half + 1) * 512], p_ps[:])
                        if half == 1:
                            m0 = m - 1
                            dst = out_d[m0 * P:(m0 + 2) * P, isl].rearrange(
                                "(two p) f -> p two f", two=2)
                            nc.sync.dma_start(dst, outc2[:])
                        return
                    outc = work.tile([P, 512], F32, name="outc")
                    nc.vector.tensor_copy(outc[:], p_ps[:])
                    # spread the writes over both rings at the end so the
                    # final drain isn't serialized on one queue
                    ring = nc.scalar if (final_i is not None and final_i % 2) \
                        else nc.sync
                    ring.dma_start(out_d[m * P:(m + 1) * P, isl], outc[:])

                def out_proj(ic, OT, ms, final=False):
                    for i, m in enumerate(ms):
                        p_ps = op_mms(ic, OT, m, 0, NH)
                        op_store(p_ps, ic, m, final_i=i if final else None)

                # ---- attention units, software-pipelined ACROSS units: the
                # S matmuls + exp of unit u+1's first two j-pairs are emitted
                # during unit u's O-drain steps (jp8/jp9), so ACT never sees a
                # unit-boundary bubble.  Fillers (Q proj, out-proj) are spread
                # one-two matmuls per jp so the per-jp PE load stays under the
                # exp pace. ----
                units = [(ic, h) for ic in range(NIC) for h in range(NH)]
                ustate = {}

                def emit_S(u, sj):
                    """S-pair sj of unit u: 2 DR matmuls + exp + fold add."""
                    ic, h = units[u]
                    st = ustate[u]
                    isl = slice(ic * 512, (ic + 1) * 512)
                    s_ps = ps_mm.tile([P, 1024], F32, name="mm")
                    for half in range(2):
                        jt = 2 * sj + half
                        jsl = slice(jt * P, (jt + 1) * P)
                        nc.tensor.matmul(
                            s_ps[:, half * 512:(half + 1) * 512],
                            KS[:, :, h, jsl], QS[:, :, h, isl],
                            start=True, stop=True, perf_mode=DR,
                        )
                    pt2 = ptp.tile([P, 1024], BF16, name="pt")
                    nc.scalar.activation(pt2[:], s_ps[:], EXP, scale=EXP_SCALE)
                    st['pt2s'][sj] = pt2
                    if sj == 1:
                        nc.vector.tensor_add(
                            st['fold'][:], st['pt2s'][0][:], st['pt2s'][1][:])
                    elif sj > 1:
                        nc.vector.tensor_add(
                            st['fold'][:], st['fold'][:], pt2[:])

                def new_unit_state(u):
                    ic, h = units[u]
                    OT = (ustate[u - 1]['OT'] if h else
                          oio.tile([P, NH, 512], BF16, name="OT"))
                    ustate[u] = dict(
                        OT=OT,
                        fold=work2.tile([P, 1024], BF16, name="fold",
                                        tag="fold"),
                        pt2s=[None] * (NJT // 2),
                    )

                pending_tail = None
                pending_proj = None
                tail_mid = None
                new_unit_state(0)
                emit_S(0, 0)
                emit_S(0, 1)
                for u, (ic, h) in enumerate(units):
                    st = ustate[u]
                    o_ps = ps_acc.tile([P, 512], F32, name="acc")
                    for jp in range(NJT // 2 + 2):
                        if jp < NJT // 2 - 2:
                            emit_S(u, jp + 2)
                        elif jp >= NJT // 2 and u + 1 < len(units):
                            if jp == NJT // 2:
                                new_unit_state(u + 1)
                            emit_S(u + 1, jp - NJT // 2)
                        if jp > 1:
                            prev = st['pt2s'][jp - 2]
                            for half in range(2):
                                jt = 2 * (jp - 2) + half
                                nc.tensor.matmul(
                                    o_ps[:], V[:, jt, h * P:(h + 1) * P],
                                    prev[:, half * 512:(half + 1) * 512],
                                    start=(jt == 0), stop=(jt == NJT - 1),
                                )
                        if ic == 0 and h == 0 and jp in (3, 5):
                            # late V token-tiles as PE filler; consumed only
                            # at this unit's last jp iterations
                            proj_v(14 if jp == 3 else 15)
                        if jp == 0 and pending_tail is not None:
                            tail_mid = norm_tail_a(pending_tail)
                            pending_tail = None
                        if jp == 2 and tail_mid is not None:
                            norm_tail_b(tail_mid)
                            tail_mid = None
                        # Q projection of the next chunk, split jp8/jp9
                        if ic < NIC - 1:
                            if jp == 8:
                                qf_ps = proj_qk_mms(h, ic + 1, QS, 0, KTP)
                            elif jp == 9:
                                proj_qk_mms(h, ic + 1, QS, KTP, 2 * KTP,
                                            q_ps=qf_ps)
                                proj_qk_store(qf_ps, ic + 1, QS, h, main=True)
                        # out-projection of the previous chunk: 2 column
                        # tiles, one-two matmuls per jp over jp3..7
                        if pending_proj is not None and 3 <= jp <= 7:
                            pic, pOT = pending_proj
                            if jp == 3:
                                op_ps = op_mms(pic, pOT, 2 * h, 0, 2)
                            elif jp == 4:
                                op_mms(pic, pOT, 2 * h, 2, 4, p_ps=op_ps)
                            elif jp == 5:
                                op_mms(pic, pOT, 2 * h, 4, 5, p_ps=op_ps)
                                outc2 = work.tile([P, 1024], F32,
                                                  name="outc2", tag="oc2")
                                op_store(op_ps, pic, 2 * h, pair=(outc2, 0))
                                op_ps = op_mms(pic, pOT, 2 * h + 1, 0, 1)
                            elif jp == 6:
                                op_mms(pic, pOT, 2 * h + 1, 1, 3, p_ps=op_ps)
                            elif jp == 7:
                                op_mms(pic, pOT, 2 * h + 1, 3, 5, p_ps=op_ps)
                                op_store(op_ps, pic, 2 * h + 1,
                                         pair=(outc2, 1))
                                if h == NH - 1:
                                    pending_proj = None
                    pending_tail = (st['fold'], o_ps, st['OT'], h)
                    if h == NH - 1:
                        pending_proj = (ic, st['OT'])
                    if u > 0:
                        del ustate[u - 1]
                norm_tail(pending_tail)
                out_proj(*pending_proj, range(D // P), final=True)

    nc.finalize()
    _PROGRAM_CACHE[repeat] = nc
    return nc


def _enc_hi_lo(a, scale):
    """Split scale*a into e4m3 hi + lo (same scale; lo holds the residual)."""
    import ml_dtypes
    f8 = ml_dtypes.float8_e4m3
    sa = np.asarray(a, np.float32) * scale
    hi = sa.astype(f8)
    lo = (sa - hi.astype(np.float32)).astype(f8)
    return hi, lo


def _shard_inputs(x, w_qkv, w_out):
    """Build the 8 per-core input maps (fp8 hi/lo operands, host-encoded)."""
    import ml_dtypes
    bf16 = ml_dtypes.bfloat16
    onesr = np.ones((1, P), np.float32)
    in_maps = []
    for c in range(8):
        b = c // 2
        h0 = NH * (c % 2)
        xT = np.ascontiguousarray(np.asarray(x[b], np.float32).T)      # [D, S]
        xh, xl = _enc_hi_lo(xT, CX)
        qk = np.concatenate([
            w_qkv[:, qi * D + h0 * HD: qi * D + (h0 + NH) * HD] for qi in range(2)
        ], axis=1)                                                     # [D, 1280]
        wqkh, _ = _enc_hi_lo(qk, CW)   # Q 2-term / K 1-term: w plain fp8
        wv = w_qkv[:, 2 * D + h0 * HD: 2 * D + (h0 + NH) * HD]         # [D, 640]
        wvh, wvl = _enc_hi_lo(wv, CW)
        in_maps.append(dict(
            xh=xh, xl=xl, wqkh=wqkh,
            wvh=wvh, wvl=wvl,
            wout=np.ascontiguousarray(
                np.asarray(w_out[h0 * HD:(h0 + NH) * HD, :], np.float32)
            ).astype(bf16),
            onesr_in=onesr,
        ))
    return in_maps


def run_sharded(x, w_qkv, w_out, b_out, repeat=1, trace=False):
    """Run the SPMD program; returns (out [B,S,D], BassKernelResults)."""
    from concourse.bass_utils import run_bass_kernel_spmd

    nc = _build_program(repeat)
    in_maps = _shard_inputs(x, w_qkv, w_out)
    res = run_bass_kernel_spmd(nc, in_maps, list(range(8)), trace=trace)
    out = np.empty((B, S, D), np.float32)
    for b in range(B):
        out[b] = (res.results[2 * b]["outT"].T
                  + res.results[2 * b + 1]["outT"].T
                  + b_out[None, :])
    return out, res


def kernel(x, w_qkv, w_out, b_out):
    x = np.asarray(x, np.float32)
    w_qkv = np.asarray(w_qkv, np.float32)
    w_out = np.asarray(w_out, np.float32)
    b_out = np.asarray(b_out, np.float32)
    out, _ = run_sharded(x, w_qkv, w_out, b_out)
    return out
